# revision 1
# baseline (speedup 1.0000x reference)
"""Trainium2 Bass kernel for nn_Attention_80384607912675.

Multi-head attention (B=2, S=2048, D=1024, H=16, HD=64), fp32 reference.

Sharding (8 cores): data-parallel over batch (2) x tensor-parallel over heads
(4 head groups of 4 heads).  Core c handles batch c//4, heads [4*(c%4), 4*(c%4)+4).
wq/wk/wv split column-wise, wo split row-wise; the wo partial sums (and the
bias bo) are reduced on the host.

Per-core kernel (all matmuls bf16 with fp32 PSUM accumulation):
  QT/KT = (x @ wq/k + b)^T  stored head-major [256, 2048]
  V_aug = [x @ wv + bv | 1] stored natural    [2048, 4*(64+1)]  (ones column
                            per head folds the softmax row-sum into PV)
  per head pair hp (A/B), q-window qw (512 wide), kp-chunk c:
    S^T[kp, q]   = K_h^T (x) Q_h   (K=64; A,B packed side-by-side in one
                                    PSUM tile, row-tiled in the PE)
    P^T          = exp(S^T / 8)    (one ScalarE instr per A|B pair, ->bf16)
    [O^T; rowsum] += V_aug^T (x) P^T   (M=65, accumulated over c)
  O^T normalized by broadcast(1/rowsum) (PE K=1 broadcast + DVE multiply)
  out_partial = O_norm^T.T @ wo_c -> [2048, 1024] bf16 (heads 0-1 staged
  under the second attention pair, heads 2-3 added in the tail)

The exp (ScalarE) stream is the bottleneck (~134us busy); all PE-side work
(projections, normalization, output projection) is interleaved into its slack
via a static filler schedule, and the lead-in QK projections pipeline against
the streamed xT DMA chunks.
"""

import numpy as np

B, S, D, H = 2, 2048, 1024, 16
HD = D // H          # 64
HPC = 4              # heads per core
DHC = HPC * HD       # 256 head dims per core
KC = D // 128        # 8 contraction chunks
SB = S // 128        # 16 s blocks / kp chunks
VP = HPC * (HD + 1)  # 260: V storage pitch per s-chunk (ones col per head)
NC = 8               # cores
NQW = 4              # 512-wide q windows per head pair

_nc_cache = {}


def _build_bass(debug=False, with_bias=False):
    import concourse.mybir as mybir
    import concourse.tile as tile
    from concourse import bacc

    BF = mybir.dt.bfloat16
    F32 = mybir.dt.float32
    EXP = mybir.ActivationFunctionType.Exp

    nc = bacc.Bacc("TRN2")

    xT_d = nc.dram_tensor("xT", [D, S], BF, kind="ExternalInput")
    wq_d = nc.dram_tensor("wq_c", [D, DHC], BF, kind="ExternalInput")
    wk_d = nc.dram_tensor("wk_c", [D, DHC], BF, kind="ExternalInput")
    wv_d = nc.dram_tensor("wv_c", [D, DHC], BF, kind="ExternalInput")
    wo_d = nc.dram_tensor("wo_c", [DHC, D], BF, kind="ExternalInput")
    bias_d = nc.dram_tensor("bias3", [1, 3 * DHC], BF, kind="ExternalInput")
    out1_d = nc.dram_tensor("out1", [S, D], BF, kind="ExternalOutput")
    out2_d = nc.dram_tensor("out2", [S, D], BF, kind="ExternalOutput")
    if debug:
        dbg = {
            "qt": nc.dram_tensor("dbg_qt", [128, 2 * S], BF, kind="ExternalOutput"),
            "kt": nc.dram_tensor("dbg_kt", [128, 2 * S], BF, kind="ExternalOutput"),
            "v": nc.dram_tensor("dbg_v", [128, SB * VP], BF, kind="ExternalOutput"),
            "oun": nc.dram_tensor(
                "dbg_oun", [HD + 1, HPC * S], F32, kind="ExternalOutput"
            ),
            "onm": nc.dram_tensor("dbg_onm", [HD, HPC * S], BF, kind="ExternalOutput"),
            "onm2": nc.dram_tensor("dbg_onm2", [128, 2 * S], BF, kind="ExternalOutput"),
            "wo2": nc.dram_tensor("dbg_wo2", [128, 2 * D], BF, kind="ExternalOutput"),
            "ost": nc.dram_tensor("dbg_ost", [128, SB * D], BF, kind="ExternalOutput"),
        }

    with tile.TileContext(nc) as tc:
        with (
            tc.tile_pool(name="persist", bufs=1) as pp,
            tc.tile_pool(name="sc", bufs=2, space="PSUM") as scp,
            tc.tile_pool(name="oacc", bufs=1, space="PSUM") as opp,
            tc.tile_pool(name="pj", bufs=2, space="PSUM") as pjp,
            tc.tile_pool(name="pt", bufs=6) as ptp,
            tc.tile_pool(name="rc", bufs=2) as rcp,
            tc.tile_pool(name="bb", bufs=4) as bbp,
            tc.tile_pool(name="osb", bufs=6) as oup,
        ):
            xT_sb = pp.tile([128, KC * S], BF, tag="xT", name="xT_sb")
            wq_sb = pp.tile([128, KC * DHC], BF, tag="wq", name="wq_sb")
            wk_sb = pp.tile([128, KC * DHC], BF, tag="wk", name="wk_sb")
            wv_sb = pp.tile([128, KC * DHC], BF, tag="wv", name="wv_sb")
            wo_sb = pp.tile([128, 2 * D], BF, tag="wo", name="wo_sb")
            qt_sb = pp.tile([128, 2 * S], BF, tag="qt", name="qt_sb")
            kt_sb = pp.tile([128, 2 * S], BF, tag="kt", name="kt_sb")
            v_sb = pp.tile([128, SB * VP], BF, tag="v", name="v_sb")
            vt_sb = pp.tile([128, 2 * S], BF, tag="vt", name="vt_sb")
            ident = pp.tile([128, 128], BF, tag="ident", name="ident")
            oun_sb = pp.tile([HD + 1, HPC * S], F32, tag="oun", name="oun_sb")
            onm_sb = pp.tile([HD, HPC * S], BF, tag="onm", name="onm_sb")
            onm2_sb = pp.tile([128, 2 * S], BF, tag="onm2", name="onm2_sb")
            bias_sb = pp.tile([1, 3 * DHC], BF, tag="bias", name="bias_sb")
            ones16 = pp.tile([1, 512], BF, tag="ones16", name="ones16")

            # input DMAs: small weights first; xT streamed in 8 chunks that the
            # lead-in projections consume as they land; wo last (needed late)
            def load_w(w_sb, w_d):
                nc.sync.dma_start(
                    w_sb[:, :].rearrange("p (k d) -> p k d", d=DHC),
                    w_d[:, :].rearrange("(k p) d -> p k d", p=128),
                )

            def load_xt(k):
                nc.sync.dma_start(
                    xT_sb[:, k * S:(k + 1) * S], xT_d[k * 128:(k + 1) * 128, :]
                )

            load_w(wq_sb, wq_d)
            for k in range(4):
                load_xt(k)
            load_w(wk_sb, wk_d)
            for k in range(4, KC):
                load_xt(k)
            load_w(wv_sb, wv_d)
            nc.sync.dma_start(bias_sb[:, :], bias_d[:, :])
            nc.sync.dma_start(
                wo_sb[:, :].rearrange("r (p d) -> r p d", d=D),
                wo_d[:, :].rearrange("(p r) d -> r p d", r=128),
            )
            nc.vector.memset(ones16[:, :], 1.0)
            # ones columns of V_aug: preset everything to 1, V overwrites below
            nc.gpsimd.memset(v_sb[:, :], 1.0)
            from concourse.masks import make_identity
            make_identity(nc, ident[:, :])

            bq = bias_sb[0:1, 0:DHC]
            bk = bias_sb[0:1, DHC:2 * DHC]
            bv = bias_sb[0:1, 2 * DHC:3 * DHC]

            def qk_mm(ps, w_sb, p, nt, k):
                nc.tensor.matmul(
                    ps[:, :],
                    lhsT=w_sb[:, k * DHC + p * 128: k * DHC + (p + 1) * 128],
                    rhs=xT_sb[:, k * S + nt * 512: k * S + (nt + 1) * 512],
                    start=(k == 0),
                    stop=(k == KC - 1 and not with_bias),
                )

            def qk_fin(ps, dst, bias, p, nt, on_act=False):
                if with_bias:
                    nc.tensor.matmul(
                        ps[:, :],
                        lhsT=bias[:, p * 128:(p + 1) * 128],
                        rhs=ones16[0:1, :],
                        start=False,
                        stop=True,
                    )
                dslice = dst[:, p * S + nt * 512: p * S + (nt + 1) * 512]
                if on_act:
                    nc.scalar.copy(dslice, ps[:, :])
                else:
                    nc.vector.tensor_copy(dslice, ps[:, :])

            _qk_pending = {}

            def proj_qk_a(dst, w_sb, bias, p, nt):
                ps = pjp.tile([128, 512], F32, tag="pj", name=f"qk_{p}_{nt}")
                for k in range(KC // 2):
                    qk_mm(ps, w_sb, p, nt, k)
                _qk_pending[(p, nt, dst.tensor.name)] = ps

            def proj_qk_b(dst, w_sb, bias, p, nt):
                ps = _qk_pending.pop((p, nt, dst.tensor.name))
                for k in range(KC // 2, KC):
                    qk_mm(ps, w_sb, p, nt, k)
                qk_fin(ps, dst, bias, p, nt)

            def proj_vt(db, nt, pool=None, tag="pj"):
                """V^T[d-block db, s-window nt]: (wv^T x^T + bv) -> vt_sb bf16."""
                ps = (pool or pjp).tile([128, 512], F32, tag=tag, name=f"vt_{db}_{nt}")
                for k in range(KC):
                    nc.tensor.matmul(
                        ps[:, :],
                        lhsT=wv_sb[:, k * DHC + db * 128: k * DHC + (db + 1) * 128],
                        rhs=xT_sb[:, k * S + nt * 512: k * S + (nt + 1) * 512],
                        start=(k == 0),
                        stop=(k == KC - 1 and not with_bias),
                    )
                if with_bias:
                    nc.tensor.matmul(
                        ps[:, :],
                        lhsT=bv[:, db * 128:(db + 1) * 128],
                        rhs=ones16[0:1, :],
                        start=False,
                        stop=True,
                    )
                nc.vector.tensor_copy(
                    vt_sb[:, db * S + nt * 512: db * S + (nt + 1) * 512], ps[:, :]
                )

            def v_tp(sb, db):
                """Transpose V^T block (d-block db, s-chunk sb) into v_sb."""
                tp = pjp.tile([128, 128], BF, tag="pj", name=f"tp_{sb}_{db}")
                nc.tensor.transpose(
                    tp[:, :], vt_sb[:, db * S + sb * 128: db * S + (sb + 1) * 128],
                    ident[:, :],
                )
                dst3 = v_sb[
                    :, sb * VP + 2 * db * (HD + 1): sb * VP + (2 * db + 2) * (HD + 1)
                ].rearrange("p (h e) -> p h e", e=HD + 1)[:, :, 0:HD]
                nc.vector.tensor_copy(dst3, tp[:, :])

            def outproj_piece(sb, n, pair, out_dram, on_act=False):
                """Half s-block head-pair partial -> bf16 -> DMA."""
                ot = oup.tile([128, 512], BF, tag="osb", name=f"ot{pair}_{sb}_{n}")
                po = pjp.tile([128, 512], F32, tag="pj", name=f"po{pair}_{sb}_{n}")
                nc.tensor.matmul(
                    po[:, :],
                    lhsT=onm2_sb[:, pair * S + sb * 128: pair * S + (sb + 1) * 128],
                    rhs=wo_sb[:, pair * D + n * 512: pair * D + (n + 1) * 512],
                    start=True,
                    stop=True,
                )
                if on_act:
                    nc.scalar.copy(ot[:, :], po[:, :])
                else:
                    nc.vector.tensor_copy(ot[:, :], po[:, :])
                nc.sync.dma_start(
                    out_dram[sb * 128:(sb + 1) * 128, n * 512:(n + 1) * 512], ot[:, :]
                )

            def outproj1(sb, n):
                outproj_piece(sb, n, 0, out1_d)

            def outproj2(sb, on_act=False):
                for n in range(2):
                    outproj_piece(sb, n, 1, out2_d, on_act=on_act)

            # ---- lead-in: QT/KT p0 all nt, k-major across 4 psum slots so the
            # matmuls pipeline against the arriving xT chunks
            lead = [
                (qt_sb, wq_sb, bq, 0, 0, pjp, "pj"),
                (kt_sb, wk_sb, bk, 0, 0, pjp, "pj"),
                (kt_sb, wk_sb, bk, 0, 1, scp, "sc"),
                (qt_sb, wq_sb, bq, 0, 1, scp, "sc"),
                (None, wv_sb, bv, 0, 0, opp, "oacc"),  # V^T(0,0)
            ]
            lead_ps = [
                pool.tile([128, 512], F32, tag=tag, name=f"lead_{nt}_{tag}")
                for dst, w_sb, bias, p, nt, pool, tag in lead
            ]
            for k in range(KC):
                for (dst, w_sb, bias, p, nt, pool, tag), ps in zip(lead, lead_ps):
                    if dst is None:
                        nc.tensor.matmul(
                            ps[:, :],
                            lhsT=wv_sb[:, k * DHC + p * 128: k * DHC + (p + 1) * 128],
                            rhs=xT_sb[:, k * S + nt * 512: k * S + (nt + 1) * 512],
                            start=(k == 0),
                            stop=(k == KC - 1 and not with_bias),
                        )
                    else:
                        qk_mm(ps, w_sb, p, nt, k)
            for (dst, w_sb, bias, p, nt, pool, tag), ps in zip(lead, lead_ps):
                if dst is None:
                    if with_bias:
                        nc.tensor.matmul(
                            ps[:, :],
                            lhsT=bv[:, p * 128:(p + 1) * 128],
                            rhs=ones16[0:1, :],
                            start=False,
                            stop=True,
                        )
                    nc.scalar.copy(
                        vt_sb[:, p * S + nt * 512: p * S + (nt + 1) * 512], ps[:, :]
                    )
                else:
                    qk_fin(ps, dst, bias, p, nt, on_act=True)

            # ---- filler schedule: (hp, qw, c) -> deferred work emitted inside
            # the ACT-bound attention loop
            fillers = {}

            def add(hp, qw, c, fn):
                fillers.setdefault((hp, qw, c), []).append(fn)

            def add_qk(hp, qw, c, dst, w_sb, bias, p, nt):
                add(hp, qw, c, lambda: proj_qk_a(dst, w_sb, bias, p, nt))
                add(hp, qw, c + 1, lambda: proj_qk_b(dst, w_sb, bias, p, nt))

            for c in range(SB):  # heads 0,1 transposes JIT (vt block c//4 ready)
                add(0, 0, c, lambda c=c: v_tp(c, 0))
            add(0, 0, 1, lambda: proj_vt(0, 1))
            add(0, 0, 5, lambda: proj_vt(0, 2))
            add(0, 0, 9, lambda: proj_vt(0, 3))
            add_qk(0, 0, 2, kt_sb, wk_sb, bk, 0, 2)
            add_qk(0, 0, 6, kt_sb, wk_sb, bk, 0, 3)
            for nt in range(4):  # V^T heads 2,3 (needed from hp1)
                add(0, 1, 4 * nt, lambda nt=nt: proj_vt(1, nt))
            for i in range(8):
                add(0, 1, 2 * i + 1, lambda sb=i: v_tp(sb, 1))
                add(0, 2, 2 * i, lambda sb=i + 8: v_tp(sb, 1))
            add_qk(0, 1, 1, qt_sb, wq_sb, bq, 0, 2)
            add_qk(0, 1, 9, qt_sb, wq_sb, bq, 0, 3)
            add_qk(0, 2, 1, kt_sb, wk_sb, bk, 1, 0)
            add_qk(0, 2, 9, kt_sb, wk_sb, bk, 1, 1)
            add_qk(0, 3, 0, kt_sb, wk_sb, bk, 1, 2)
            add_qk(0, 3, 4, kt_sb, wk_sb, bk, 1, 3)
            add_qk(0, 3, 8, qt_sb, wq_sb, bq, 1, 0)
            add_qk(0, 3, 12, qt_sb, wq_sb, bq, 1, 1)
            add_qk(1, 0, 2, qt_sb, wq_sb, bq, 1, 2)
            add_qk(1, 0, 6, qt_sb, wq_sb, bq, 1, 3)
            for i in range(32):  # outproj stage 1 spread over hp1 qw0/qw1
                sb, n = divmod(i, 2)
                add(1, i // 16, i % 16, lambda sb=sb, n=n: outproj1(sb, n))
            for i in range(8):  # outproj stage 2 for sb 0..7 under hp1 qw2
                add(1, 2, 1 + 2 * (i % 8), lambda sb=i: outproj2(sb))
            for i in range(8, 12):  # sb 8..11 under hp1 qw3 (after its drains)
                add(1, 3, 4 + 2 * (i - 8), lambda sb=i: outproj2(sb))

            def drain_window(hp, qw, oacc, part):
                """Deferred per-window drain: park+recip / bcast+norm+relocate."""
                hA = 2 * hp
                oun4 = oun_sb[:, :].rearrange("p (h s) -> p h s", h=HPC)
                if part == 0:
                    nc.vector.tensor_copy(
                        oun4[0:HD + 1, hA:hA + 2, qw * 512:(qw + 1) * 512],
                        oacc[:, :],
                    )
                    return
                rs0 = rcp.tile([1, 1024], F32, tag="rs0", name=f"rs0_{hp}{qw}")
                nc.sync.dma_start(
                    rs0[0:1, :],
                    oun4[HD:HD + 1, hA:hA + 2, qw * 512:(qw + 1) * 512],
                )
                rc = rcp.tile([1, 1024], F32, tag="rc", name=f"rc_{hp}{qw}")
                nc.vector.reciprocal_approx_fast(out=rc[0:1, :], in_=rs0[0:1, :])
                for i in range(2):
                    h = 2 * hp + i
                    pb = bbp.tile([HD, 512], F32, tag="bb", name=f"bb_{hp}{qw}{i}")
                    nc.gpsimd.partition_broadcast(
                        pb[:, :], rc[0:1, i * 512:(i + 1) * 512]
                    )
                    qcol = h * S + qw * 512
                    nc.vector.tensor_mul(
                        onm_sb[0:HD, qcol:qcol + 512],
                        oun_sb[0:HD, qcol:qcol + 512],
                        pb[:, :],
                    )
                    nc.sync.dma_start(
                        onm2_sb[64 * i:64 * (i + 1), hp * S + qw * 512:
                                hp * S + (qw + 1) * 512],
                        onm_sb[0:HD, qcol:qcol + 512],
                    )

            # ---- attention
            pending_drain = []
            for hp in range(2):
                for qw in range(NQW):
                    oacc = opp.tile(
                        [HD + 1, 1024], F32, tag="oacc", name=f"o_{hp}_{qw}"
                    )
                    prev = None

                    def emit_pv(pt_t, c, oacc=oacc, hp=hp):
                        for i in range(2):
                            nc.tensor.matmul(
                                oacc[:, i * 512:(i + 1) * 512],
                                lhsT=v_sb[
                                    :, c * VP + (HD + 1) * (2 * hp + i):
                                    c * VP + (HD + 1) * (2 * hp + i + 1)
                                ],
                                rhs=pt_t[:, 512 * i:512 * (i + 1)],
                                start=(c == 0),
                                stop=(c == SB - 1),
                            )

                    for c in range(SB):
                        sc = scp.tile(
                            [128, 1024], F32, tag="sc", name=f"sc_{hp}{qw}{c}"
                        )
                        for i in range(2):  # head A | head B packed
                            nc.tensor.matmul(
                                sc[:, 512 * i:512 * (i + 1)],
                                lhsT=kt_sb[
                                    64 * i:64 * (i + 1),
                                    hp * S + c * 128: hp * S + (c + 1) * 128,
                                ],
                                rhs=qt_sb[
                                    64 * i:64 * (i + 1),
                                    hp * S + qw * 512: hp * S + (qw + 1) * 512,
                                ],
                                start=True,
                                stop=True,
                            )
                        pt_t = ptp.tile(
                            [128, 1024], BF, tag="pt", name=f"pt_{hp}{qw}{c}"
                        )
                        nc.scalar.activation(pt_t[:, :], sc[:, :], EXP, scale=0.125)
                        if pending_drain and c == 0:
                            drain_window(*pending_drain[0], 0)
                        elif pending_drain and c == 2:
                            drain_window(*pending_drain.pop(0), 1)
                        for fn in fillers.get((hp, qw, c), ()):
                            fn()
                        if prev is not None:
                            emit_pv(prev, c - 1)
                        prev = pt_t
                    emit_pv(prev, SB - 1)
                    pending_drain.append((hp, qw, oacc))

            while pending_drain:
                hp, qw, oacc = pending_drain.pop(0)
                drain_window(hp, qw, oacc, 0)
                drain_window(hp, qw, oacc, 1)
            for sb in range(12, SB):
                outproj2(sb, on_act=True)

            if debug:
                nc.sync.dma_start(dbg["qt"][:, :], qt_sb[:, :])
                nc.sync.dma_start(dbg["kt"][:, :], kt_sb[:, :])
                nc.sync.dma_start(dbg["v"][:, :], v_sb[:, :])
                nc.sync.dma_start(dbg["oun"][:, :], oun_sb[:, :])
                nc.sync.dma_start(dbg["onm"][:, :], onm_sb[:, :])
                nc.sync.dma_start(dbg["onm2"][:, :], onm2_sb[:, :])
                nc.sync.dma_start(dbg["wo2"][:, :], wo_sb[:, :])
                nc.sync.dma_start(dbg["ost"][:, :], ost_sb[:, :])

    nc.compile()
    return nc


def _get_nc(with_bias=False):
    if with_bias not in _nc_cache:
        _nc_cache[with_bias] = _build_bass(with_bias=with_bias)
    return _nc_cache[with_bias]


def _prepare_in_maps(x, wq, bq, wk, bk, wv, bv, wo):
    import ml_dtypes

    bf16 = ml_dtypes.bfloat16
    x = np.asarray(x, np.float32)
    wq, bq = np.asarray(wq, np.float32), np.asarray(bq, np.float32)
    wk, bk = np.asarray(wk, np.float32), np.asarray(bk, np.float32)
    wv, bv = np.asarray(wv, np.float32), np.asarray(bv, np.float32)
    wo = np.asarray(wo, np.float32)

    xT = [np.ascontiguousarray(x[b].T).astype(bf16) for b in range(B)]
    in_maps = []
    for c in range(NC):
        b, j = divmod(c, HPC)
        cs = slice(DHC * j, DHC * (j + 1))
        bias3 = np.concatenate([bq[cs], bk[cs], bv[cs]]).reshape(1, 3 * DHC).astype(bf16)
        in_maps.append(
            {
                "xT": xT[b],
                "wq_c": np.ascontiguousarray(wq[:, cs]).astype(bf16),
                "wk_c": np.ascontiguousarray(wk[:, cs]).astype(bf16),
                "wv_c": np.ascontiguousarray(wv[:, cs]).astype(bf16),
                "wo_c": np.ascontiguousarray(wo[cs, :]).astype(bf16),
                "bias3": np.ascontiguousarray(bias3),
            }
        )
    return in_maps


def _gather(parts, bo):
    bo = np.asarray(bo, np.float32)
    out = np.empty((B, S, D), np.float32)
    for b in range(B):
        acc = np.asarray(parts[HPC * b], np.float32)
        for j in range(1, HPC):
            acc = acc + np.asarray(parts[HPC * b + j], np.float32)
        out[b] = acc + bo
    return out


def kernel(x, wq, bq, wk, bk, wv, bv, wo, bo):
    from concourse import bass_utils

    in_maps = _prepare_in_maps(x, wq, bq, wk, bk, wv, bv, wo)
    with_bias = bool(
        np.any(np.asarray(bq)) or np.any(np.asarray(bk)) or np.any(np.asarray(bv))
    )
    res = bass_utils.run_bass_kernel_spmd(
        nc=_get_nc(with_bias), in_maps=in_maps, core_ids=list(range(NC))
    )
    parts = [
        np.asarray(r["out1"], np.float32) + np.asarray(r["out2"], np.float32)
        for r in res.results
    ]
    return _gather(parts, bo)



# revision 8
# speedup vs baseline: 1.2132x; 1.2132x over previous
"""Trainium2 Bass kernel for nn_Attention_80384607912675.

Multi-head attention (B=2, S=2048, D=1024, H=16, HD=64), fp32 reference,
bf16 on-chip compute.

Sharding (8 cores): data-parallel over batch (2) x tensor-parallel over heads
(4 head groups of 4 heads).  Core c handles batch c//4, heads [4*(c%4), 4*(c%4)+4).
wq/wk/wv split column-wise, wo split row-wise; the wo partials (4 per batch)
and bias bo are reduced on the host.

Per-core structure (head pairs hp in {0,1}, 512-wide q windows qw, kp chunks c):
  QT/KT  = (x @ wq/k)^T head-major [128, 2048] per pair     (wq chunk stationary)
  V      = x @ wv natural [s, 4*(64+1)] per s-chunk          (xT chunk stationary;
           ones column per head folds the softmax row-sum into PV)
  S^T[kp, q] = K_h^T (x) Q_h            (K=64, kt chunk stationary, N=512)
  P^T    = exp(S^T / 8)  -> bf16        (one ScalarE instr per (hp,qw,c))
  O_nat[q, h*65] += P^T-slice (x) V_aug (P STATIONARY, N=65: 2x fewer PE
                                         cycles than V-stationary)
  normalize: 1/rowsum via DVE reciprocal_approx_fast, per-partition
             tensor_scalar_mul -> onm natural bf16
  onm^T via DMA-transpose engine (no PE cycles)
  out[q, :] = onm^T-slices @ wo (both pairs accumulated) -> single output

The exp stream on ScalarE (~133us) and the PE matmul stream (~137us) are both
near-critical; all projection/outproj work is interleaved into the attention
loop by a deadline-driven scheduler so neither engine starves.
"""

import numpy as np

B, S, D, H = 2, 2048, 1024, 16
HD = D // H          # 64
HPC = 4              # heads per core
DHC = HPC * HD       # 256 head dims per core
KC = D // 128        # 8 contraction chunks
SB = S // 128        # 16 s blocks / kp chunks
VP = HPC * (HD + 1)  # 260: V storage pitch per s-chunk (ones col per head)
NC = 8               # cores
NQW = 4              # 512-wide q windows per head pair

_nc_cache = {}


def _build_bass(with_bias=False, debug=False):
    import concourse.mybir as mybir
    import concourse.tile as tile
    from concourse import bacc

    BF = mybir.dt.bfloat16
    F32 = mybir.dt.float32
    EXP = mybir.ActivationFunctionType.Exp

    nc = bacc.Bacc("TRN2")

    xT_d = nc.dram_tensor("xT", [D, S], BF, kind="ExternalInput")
    wq_d = nc.dram_tensor("wq_c", [D, DHC], BF, kind="ExternalInput")
    wk_d = nc.dram_tensor("wk_c", [D, DHC], BF, kind="ExternalInput")
    wv_d = nc.dram_tensor("wv_c", [D, DHC], BF, kind="ExternalInput")
    wo_d = nc.dram_tensor("wo_c", [DHC, D], BF, kind="ExternalInput")
    if with_bias:
        bias_d = nc.dram_tensor("bias3", [1, 3 * DHC], BF, kind="ExternalInput")
    out_d = nc.dram_tensor("out", [S, D], BF, kind="ExternalOutput")
    if debug:
        dbg = {
            "qt": nc.dram_tensor("dbg_qt", [128, 2 * S], BF, kind="ExternalOutput"),
            "kt": nc.dram_tensor("dbg_kt", [128, 2 * S], BF, kind="ExternalOutput"),
            "v": nc.dram_tensor("dbg_v", [128, SB * VP], BF, kind="ExternalOutput"),
            "onm": nc.dram_tensor("dbg_onm", [128, 32 * 128], BF, kind="ExternalOutput"),
            "onmT": nc.dram_tensor("dbg_onmT", [128, 2 * S], BF, kind="ExternalOutput"),
        }

    with tile.TileContext(nc) as tc:
        with (
            tc.tile_pool(name="persist", bufs=1) as pp,
            tc.tile_pool(name="sc", bufs=2, space="PSUM") as scp,
            tc.tile_pool(name="oacc", bufs=1, space="PSUM") as oap,
            tc.tile_pool(name="pj", bufs=2, space="PSUM") as pjp,
            tc.tile_pool(name="pt", bufs=16) as ptp,
            tc.tile_pool(name="rc", bufs=4) as rcp,
            tc.tile_pool(name="osb", bufs=4) as oup,
        ):
            xT_sb = pp.tile([128, KC * S], BF, tag="xT", name="xT_sb")
            wq_sb = pp.tile([128, KC * DHC], BF, tag="wq", name="wq_sb")
            wk_sb = pp.tile([128, KC * DHC], BF, tag="wk", name="wk_sb")
            wv_sb = pp.tile([128, KC * DHC], BF, tag="wv", name="wv_sb")
            wo_sb = pp.tile([128, 2 * D], BF, tag="wo", name="wo_sb")
            qt_sb = pp.tile([128, 2 * S], BF, tag="qt", name="qt_sb")
            kt_sb = pp.tile([128, 2 * S], BF, tag="kt", name="kt_sb")
            v_sb = pp.tile([128, SB * VP], BF, tag="v", name="v_sb")
            onm_sb = pp.tile([128, 32 * 128], BF, tag="onm", name="onm_sb")
            onmT_sb = pp.tile([128, 2 * S], BF, tag="onmT", name="onmT_sb")
            if with_bias:
                bias_sb = pp.tile([1, 3 * DHC], BF, tag="bias", name="bias_sb")
                ones16 = pp.tile([1, 512], BF, tag="ones16", name="ones16")

            # ---- input DMAs.  wq/wk first (gate the lead-in projections);
            # xT streamed in s-quarters, later quarters split between the SP
            # HWDGE queue and the Pool SWDGE queue to beat the per-DMA HWDGE
            # overhead; wv after quarter 0; wo last.
            def load_w(w_sb, w_d):
                nc.sync.dma_start(
                    w_sb[:, :].rearrange("p (k d) -> p k d", d=DHC),
                    w_d[:, :].rearrange("(k p) d -> p k d", p=128),
                )

            def load_xt_q(k, nt, engine):
                engine.dma_start(
                    xT_sb[:, k * S + nt * 512: k * S + (nt + 1) * 512],
                    xT_d[k * 128:(k + 1) * 128, nt * 512:(nt + 1) * 512],
                )

            load_w(wq_sb, wq_d)
            load_w(wk_sb, wk_d)
            if with_bias:
                nc.sync.dma_start(bias_sb[:, :], bias_d[:, :])
            for k in range(KC):
                load_xt_q(k, 0, nc.sync)
            load_w(wv_sb, wv_d)
            for nt in range(1, 4):
                for k in range(KC):
                    load_xt_q(k, nt, nc.sync if k < 4 else nc.gpsimd)
            nc.sync.dma_start(
                wo_sb[:, :].rearrange("r (p d) -> r p d", d=D),
                wo_d[:, :].rearrange("(p r) d -> r p d", r=128),
            )
            # ones columns of V_aug: preset everything to 1, V overwrites below
            nc.gpsimd.memset(v_sb[:, :], 1.0)
            if with_bias:
                nc.vector.memset(ones16[:, :], 1.0)
                bq = bias_sb[0:1, 0:DHC]
                bk = bias_sb[0:1, DHC:2 * DHC]
                bv = bias_sb[0:1, 2 * DHC:3 * DHC]

            # ================= projection emitters =================
            def qk_mm(ps, w_sb, p, nt, k):
                nc.tensor.matmul(
                    ps[:, :],
                    lhsT=w_sb[:, k * DHC + p * 128: k * DHC + (p + 1) * 128],
                    rhs=xT_sb[:, k * S + nt * 512: k * S + (nt + 1) * 512],
                    start=(k == 0),
                    stop=(k == KC - 1 and not with_bias),
                )

            def qk_fin(ps, dst, bias, p, nt, on_act=False):
                if with_bias:
                    nc.tensor.matmul(
                        ps[:, :],
                        lhsT=bias[:, p * 128:(p + 1) * 128],
                        rhs=ones16[0:1, :],
                        start=False,
                        stop=True,
                    )
                dslice = dst[:, p * S + nt * 512: p * S + (nt + 1) * 512]
                if on_act:
                    nc.scalar.copy(dslice, ps[:, :])
                else:
                    nc.vector.tensor_copy(dslice, ps[:, :])

            def v_mm(ps, sb, pair, k):
                # V natural: out[s,dhc] = sum_d x[s,d] wv[d,dhc]; xT chunk stationary
                nc.tensor.matmul(
                    ps[:, :],
                    lhsT=xT_sb[:, k * S + sb * 128: k * S + (sb + 1) * 128],
                    rhs=wv_sb[:, k * DHC + pair * 128: k * DHC + (pair + 1) * 128],
                    start=(k == 0),
                    stop=(k == KC - 1 and not with_bias),
                )

            def v_fin(ps, sb, pair):
                if with_bias:
                    nc.tensor.matmul(
                        ps[:, :],
                        lhsT=ones16[0:1, 0:128],
                        rhs=bv[0:1, pair * 128:(pair + 1) * 128],
                        start=False,
                        stop=True,
                    )
                dst = v_sb[
                    :, sb * VP + 2 * pair * (HD + 1): sb * VP + (2 * pair + 2) * (HD + 1)
                ].rearrange("p (h e) -> p h e", e=HD + 1)[:, :, 0:HD]
                src = ps[:, :].rearrange("p (h e) -> p h e", e=HD)
                nc.vector.tensor_copy(dst, src)

            # ================= filler unit list =================
            # Each unit: (deadline_slot, tile_key, emit_fn, cost_ns).
            # Units of one tile must stay consecutive w.r.t. the pj pool ring;
            # the scheduler emits units in list order gated by deadline/budget.
            units = []
            v_emitted = [[False] * SB, [False] * SB]  # [pair][sb]

            def add_qk_tile(dl, dst, w_sb, bias, p, nt):
                state = {}

                def mk(kk):
                    def f():
                        if kk == 0:
                            state["ps"] = pjp.tile(
                                [128, 512], F32, tag="pj", name=f"qk{p}{nt}"
                            )
                        qk_mm(state["ps"], w_sb, p, nt, kk)
                        qk_mm(state["ps"], w_sb, p, nt, kk + 1)
                        if kk == KC - 2:
                            qk_fin(state["ps"], dst, bias, p, nt)
                    return f

                for kk in range(0, KC, 2):
                    units.append([dl, mk(kk), 430])

            def add_v_tile(dl, sb, pair):
                state = {}

                def mk(kk):
                    def f():
                        if kk == 0:
                            state["ps"] = pjp.tile(
                                [128, 128], F32, tag="pj", name=f"v{sb}_{pair}"
                            )
                        for k2 in range(kk, kk + 4):
                            v_mm(state["ps"], sb, pair, k2)
                        if kk == KC - 4:
                            v_fin(state["ps"], sb, pair)
                            v_emitted[pair][sb] = True
                    return f

                for kk in range(0, KC, 4):
                    units.append([dl, mk(kk), 220, ("v", pair, sb)])

            bq_ = bk_ = None
            if with_bias:
                bq_, bk_ = bq, bk
            # deadlines in body-slot units (slot = (hp*4+qw)*16 + c)
            add_qk_tile(2, kt_sb, wk_sb, bk_, 0, 1)
            add_qk_tile(6, kt_sb, wk_sb, bk_, 0, 2)
            add_qk_tile(10, kt_sb, wk_sb, bk_, 0, 3)
            add_qk_tile(13, qt_sb, wq_sb, bq_, 0, 1)
            for sb in range(SB):
                add_v_tile(max(2, 6 + sb), sb, 0)
            add_qk_tile(28, qt_sb, wq_sb, bq_, 0, 2)
            add_qk_tile(44, qt_sb, wq_sb, bq_, 0, 3)
            add_qk_tile(54, kt_sb, wk_sb, bk_, 1, 0)
            add_qk_tile(58, qt_sb, wq_sb, bq_, 1, 0)
            add_qk_tile(62, kt_sb, wk_sb, bk_, 1, 1)
            add_qk_tile(68, kt_sb, wk_sb, bk_, 1, 2)
            add_qk_tile(72, kt_sb, wk_sb, bk_, 1, 3)
            add_qk_tile(76, qt_sb, wq_sb, bq_, 1, 1)
            for sb in range(SB):
                add_v_tile(78 + sb, sb, 1)
            add_qk_tile(92, qt_sb, wq_sb, bq_, 1, 2)
            add_qk_tile(106, qt_sb, wq_sb, bq_, 1, 3)

            # outproj: gated on onmT of BOTH pairs for the tile's q window.
            op_ready = [[False, False] for _ in range(NQW)]  # [qw][hp]

            def add_outproj(sb):
                qw = sb // 4
                state = {}

                def mkmm(n):
                    def f():
                        ps = pjp.tile([128, 512], F32, tag="pj", name=f"op{sb}_{n}")
                        state[n] = ps
                        if n == 0:
                            state["ot"] = oup.tile(
                                [128, 1024], BF, tag="osb", name=f"ot{sb}"
                            )
                        for p in range(2):
                            nc.tensor.matmul(
                                ps[:, :],
                                lhsT=onmT_sb[:, p * S + sb * 128:p * S + (sb + 1) * 128],
                                rhs=wo_sb[:, p * D + n * 512: p * D + (n + 1) * 512],
                                start=(p == 0),
                                stop=(p == 1),
                            )
                        nc.vector.tensor_copy(
                            state["ot"][:, n * 512:(n + 1) * 512], ps[:, :]
                        )
                        if n == 1:
                            nc.sync.dma_start(
                                out_d[sb * 128:(sb + 1) * 128, :], state["ot"][:, :]
                            )
                    return f

                for n in range(2):
                    units.append(
                        [64 + 16 * qw + 24, mkmm(n), 520, ("op", qw)]
                    )

            for sb in range(SB):
                add_outproj(sb)

            units.sort(key=lambda u: u[0])

            # ================= attention =================
            def emit_pv(hp, qw, c, pt_t, oA, oB):
                # PSUM `start` marks the whole 2KB bank pending-zero, so only
                # the FIRST strip written into each oacc bank may carry it;
                # the other strips' first write then overwrites (pending) and
                # later chunks accumulate.
                for i in range(2):
                    for qb in range(4):
                        oacc = oA if qb < 2 else oB
                        loc = (qb % 2) * 2 + i
                        nc.tensor.matmul(
                            oacc[:, loc * 65: loc * 65 + 65],
                            lhsT=pt_t[:, i * 512 + qb * 128: i * 512 + (qb + 1) * 128],
                            rhs=v_sb[
                                :, c * VP + (2 * hp + i) * 65:
                                c * VP + (2 * hp + i) * 65 + 65
                            ],
                            start=(c == 0 and loc == 0),
                            stop=(c == SB - 1),
                            skip_group_check=True,
                        )

            def emit_drain(hp, qw, oA, oB):
                rcs = []
                for t, oacc in ((0, oA), (1, oB)):
                    rc = rcp.tile([128, 4], F32, tag="rc", name=f"rc{hp}{qw}{t}")
                    nc.vector.reciprocal_approx_fast(
                        out=rc[:, :].rearrange("p (s e) -> p s e", e=1),
                        in_=oacc[:, :].rearrange("p (s e) -> p s e", e=65)[:, :, 64:65],
                    )
                    rcs.append(rc)
                for qb in range(4):
                    oacc = oA if qb < 2 else oB
                    rc = rcs[0] if qb < 2 else rcs[1]
                    blk = ((hp * 4 + qw) * 4 + qb) * 128
                    for i in range(2):
                        loc = (qb % 2) * 2 + i
                        nc.vector.tensor_scalar_mul(
                            onm_sb[:, blk + i * 64: blk + (i + 1) * 64],
                            oacc[:, loc * 65: loc * 65 + 64],
                            rc[:, loc:loc + 1],
                        )
                for qb in range(4):
                    blk = ((hp * 4 + qw) * 4 + qb) * 128
                    nc.sync.dma_start(
                        onmT_sb[:, hp * S + qw * 512 + qb * 128:
                                hp * S + qw * 512 + (qb + 1) * 128],
                        onm_sb[:, blk:blk + 128],
                        transpose=True,
                    )
                op_ready[qw][hp] = True

            # ---- lead-in: QT/KT (pair 0, window 0) pipelined against the
            # arriving xT quarter-0 chunks; fins on ACT (idle before attention)
            lead = [
                (qt_sb, wq_sb, bq_, 0, 0),
                (kt_sb, wk_sb, bk_, 0, 0),
            ]
            lead_ps = [
                pjp.tile([128, 512], F32, tag="pj", name=f"lead_{i}")
                for i in range(2)
            ]
            for k in range(KC):
                for (dst, w_sb, bias, p, nt), ps in zip(lead, lead_ps):
                    qk_mm(ps, w_sb, p, nt, k)
            for (dst, w_sb, bias, p, nt), ps in zip(lead, lead_ps):
                qk_fin(ps, dst, bias, p, nt, on_act=True)

            # ---- main loop
            ui = 0           # next filler unit
            pending = []     # FIFO of (hp, qw, c, pt_tile)
            oacc_cur = {}    # (hp,qw) -> (oA, oB)
            drained = set()

            def get_oacc(hp, qw):
                if (hp, qw) not in oacc_cur:
                    oacc_cur[(hp, qw)] = (
                        oap.tile([128, 260], F32, tag="oA", name=f"oA{hp}{qw}"),
                        oap.tile([128, 260], F32, tag="oB", name=f"oB{hp}{qw}"),
                    )
                return oacc_cur[(hp, qw)]

            def pv_head_ready():
                hp_, qw_, c_, _ = pending[0]
                return v_emitted[hp_][c_]

            def force_v(pair, sb):
                """Emit all remaining units of V tile (pair, sb) immediately.
                Out-of-order vs the deadline list is fine for the pj ring as
                long as tile-internal unit order is preserved."""
                j = ui
                while not v_emitted[pair][sb]:
                    assert j < len(units), f"no units left for V({pair},{sb})"
                    u = units[j]
                    if len(u) > 3 and u[3] == ("v", pair, sb):
                        u[1]()
                        units.pop(j)
                    else:
                        j += 1

            def pop_pv(maxn):
                n = 0
                while pending and n < maxn:
                    if not pv_head_ready():
                        break
                    hp_, qw_, c_, pt_ = pending.pop(0)
                    oA, oB = get_oacc(hp_, qw_)
                    emit_pv(hp_, qw_, c_, pt_, oA, oB)
                    n += 1
                    if c_ == SB - 1:
                        emit_drain(hp_, qw_, oA, oB)
                        drained.add((hp_, qw_))
                        del oacc_cur[(hp_, qw_)]

            budget_rate = 430.0  # ns of filler PE time per slot, avg
            budget = -6000.0     # lead-in already consumed PE time

            for hp in range(2):
                for qw in range(NQW):
                    for c in range(SB):
                        slot = (hp * NQW + qw) * SB + c
                        # scores for chunk c (both heads packed)
                        sc = scp.tile([128, 1024], F32, tag="sc", name=f"s{slot}")
                        for i in range(2):
                            nc.tensor.matmul(
                                sc[:, 512 * i:512 * (i + 1)],
                                lhsT=kt_sb[
                                    64 * i:64 * (i + 1),
                                    hp * S + c * 128: hp * S + (c + 1) * 128,
                                ],
                                rhs=qt_sb[
                                    64 * i:64 * (i + 1),
                                    hp * S + qw * 512: hp * S + (qw + 1) * 512,
                                ],
                                start=True,
                                stop=True,
                            )
                        # keep the pt ring from stalling the exp stream
                        if len(pending) >= 14:
                            hp_, qw_, c_, _ = pending[0]
                            if not v_emitted[hp_][c_]:
                                force_v(hp_, c_)
                            pop_pv(4)
                        pt_t = ptp.tile([128, 1024], BF, tag="pt", name=f"p{slot}")
                        nc.scalar.activation(pt_t[:, :], sc[:, :], EXP, scale=0.125)
                        pending.append((hp, qw, c, pt_t))
                        pop_pv(2 if len(pending) > 6 else 1)
                        # fillers by deadline + budget
                        budget += budget_rate
                        while ui < len(units):
                            u = units[ui]
                            if u[0] > slot and budget < u[2]:
                                break
                            if len(u) > 3 and u[3][0] == "op":
                                qw_ = u[3][1]
                                if not (op_ready[qw_][0] and op_ready[qw_][1]):
                                    break
                            u[1]()
                            budget -= u[2]
                            ui += 1

            # ---- tail
            while pending:
                if not pv_head_ready():
                    hp_, qw_, c_, _ = pending[0]
                    force_v(hp_, c_)
                pop_pv(99)
            while ui < len(units):
                u = units[ui]
                if len(u) > 3 and u[3][0] == "op":
                    qw_ = u[3][1]
                    assert op_ready[qw_][0] and op_ready[qw_][1]
                u[1]()
                ui += 1

            if debug:
                nc.sync.dma_start(dbg["qt"][:, :], qt_sb[:, :])
                nc.sync.dma_start(dbg["kt"][:, :], kt_sb[:, :])
                nc.sync.dma_start(dbg["v"][:, :], v_sb[:, :])
                nc.sync.dma_start(dbg["onm"][:, :], onm_sb[:, :])
                nc.sync.dma_start(dbg["onmT"][:, :], onmT_sb[:, :])

    nc.compile()
    return nc


def _get_nc(with_bias=False):
    if with_bias not in _nc_cache:
        _nc_cache[with_bias] = _build_bass(with_bias=with_bias)
    return _nc_cache[with_bias]


def _prepare_in_maps(x, wq, bq, wk, bk, wv, bv, wo, with_bias):
    import ml_dtypes

    bf16 = ml_dtypes.bfloat16
    x = np.asarray(x, np.float32)
    wq, bq = np.asarray(wq, np.float32), np.asarray(bq, np.float32)
    wk, bk = np.asarray(wk, np.float32), np.asarray(bk, np.float32)
    wv, bv = np.asarray(wv, np.float32), np.asarray(bv, np.float32)
    wo = np.asarray(wo, np.float32)

    xT = [np.ascontiguousarray(x[b].T).astype(bf16) for b in range(B)]
    in_maps = []
    for c in range(NC):
        b, j = divmod(c, HPC)
        cs = slice(DHC * j, DHC * (j + 1))
        m = {
            "xT": xT[b],
            "wq_c": np.ascontiguousarray(wq[:, cs]).astype(bf16),
            "wk_c": np.ascontiguousarray(wk[:, cs]).astype(bf16),
            "wv_c": np.ascontiguousarray(wv[:, cs]).astype(bf16),
            "wo_c": np.ascontiguousarray(wo[cs, :]).astype(bf16),
        }
        if with_bias:
            bias3 = np.concatenate([bq[cs], bk[cs], bv[cs]]).reshape(1, 3 * DHC)
            m["bias3"] = np.ascontiguousarray(bias3.astype(bf16))
        in_maps.append(m)
    return in_maps


def kernel(x, wq, bq, wk, bk, wv, bv, wo, bo):
    from concourse import bass_utils

    with_bias = bool(
        np.any(np.asarray(bq)) or np.any(np.asarray(bk)) or np.any(np.asarray(bv))
    )
    in_maps = _prepare_in_maps(x, wq, bq, wk, bk, wv, bv, wo, with_bias)
    res = bass_utils.run_bass_kernel_spmd(
        nc=_get_nc(with_bias), in_maps=in_maps, core_ids=list(range(NC))
    )
    bo = np.asarray(bo, np.float32)
    out = np.empty((B, S, D), np.float32)
    for b in range(B):
        acc = np.asarray(res.results[HPC * b]["out"], np.float32)
        for j in range(1, HPC):
            acc = acc + np.asarray(res.results[HPC * b + j]["out"], np.float32)
        out[b] = acc + bo
    return out


# revision 36
# speedup vs baseline: 1.2234x; 1.0084x over previous
"""Trainium2 Bass kernel for nn_Attention_80384607912675.

Multi-head attention (B=2, S=2048, D=1024, H=16, HD=64), fp32 reference,
bf16 on-chip compute.

Sharding (8 cores): data-parallel over batch (2) x tensor-parallel over heads
(4 head groups of 4 heads).  Core c handles batch c//4, heads [4*(c%4), 4*(c%4)+4).
wq/wk/wv split column-wise, wo split row-wise; the wo partials (4 per batch)
and bias bo are reduced on the host.

Per-core structure (head pairs hp in {0,1}, 512-wide q windows qw, kp chunks c):
  QT/KT  = (x @ wq/k)^T head-major [128, 2048] per pair     (wq chunk stationary)
  V      = x @ wv natural [s, 4*(64+1)] per s-chunk          (xT chunk stationary;
           ones column per head folds the softmax row-sum into PV)
  S^T[kp, q] = K_h^T (x) Q_h            (K=64, kt chunk stationary, N=512)
  P^T    = exp(S^T / 8)  -> bf16        (one ScalarE instr per (hp,qw,c))
  O_nat[q, h*65] += P^T-slice (x) V_aug (P STATIONARY, N=65: 2x fewer PE
                                         cycles than V-stationary)
  normalize: 1/rowsum via DVE reciprocal_approx_fast, per-partition
             tensor_scalar_mul -> onm natural bf16
  onm^T via DMA-transpose engine (no PE cycles)
  out[q, :] = onm^T-slices @ wo (both pairs accumulated) -> single output

The exp stream on ScalarE (~133us) and the PE matmul stream (~137us) are both
near-critical; all projection/outproj work is interleaved into the attention
loop by a deadline-driven scheduler so neither engine starves.
"""

import numpy as np
import os
_K = lambda n, d: type(d)(os.environ.get(n, d))

B, S, D, H = 2, 2048, 1024, 16
HD = D // H          # 64
HPC = 4              # heads per core
DHC = HPC * HD       # 256 head dims per core
KC = D // 128        # 8 contraction chunks
SB = S // 128        # 16 s blocks / kp chunks
VP = HPC * (HD + 1)  # 260: V storage pitch per s-chunk (ones col per head)
NC = 8               # cores
NQW = 4              # 512-wide q windows per head pair

_nc_cache = {}


def _build_bass(with_bias=False, debug=False):
    PTBUFS = _K('PTBUFS', 16)
    import concourse.mybir as mybir
    import concourse.tile as tile
    from concourse import bacc

    BF = mybir.dt.bfloat16
    F32 = mybir.dt.float32
    EXP = mybir.ActivationFunctionType.Exp

    nc = bacc.Bacc("TRN2")

    xT_d = nc.dram_tensor("xT", [D, S], BF, kind="ExternalInput")
    # wq/wk/wv arrive host-swizzled as [2 halves * 128 cols, 8 chunks * 128 d]
    # so each half loads as one contiguous [128, 1024] DMA (no small-run 2x
    # descriptor penalty).
    wq_d = nc.dram_tensor("wq_c", [DHC, D], BF, kind="ExternalInput")
    wk_d = nc.dram_tensor("wk_c", [DHC, D], BF, kind="ExternalInput")
    wv_d = nc.dram_tensor("wv_c", [DHC, D], BF, kind="ExternalInput")
    wo_d = nc.dram_tensor("wo_c", [DHC, D], BF, kind="ExternalInput")
    if with_bias:
        bias_d = nc.dram_tensor("bias3", [1, 3 * DHC], BF, kind="ExternalInput")
    out_d = nc.dram_tensor("out", [S, D], BF, kind="ExternalOutput")
    if debug:
        dbg = {
            "qt": nc.dram_tensor("dbg_qt", [128, 2 * S], BF, kind="ExternalOutput"),
            "kt": nc.dram_tensor("dbg_kt", [128, 2 * S], BF, kind="ExternalOutput"),
            "v": nc.dram_tensor("dbg_v", [128, SB * VP], BF, kind="ExternalOutput"),
            "onm": nc.dram_tensor("dbg_onm", [128, 32 * 128], BF, kind="ExternalOutput"),
            "onmT": nc.dram_tensor("dbg_onmT", [128, 2 * S], BF, kind="ExternalOutput"),
        }

    with tile.TileContext(nc) as tc:
        with (
            tc.tile_pool(name="persist", bufs=1) as pp,
            tc.tile_pool(name="sc", bufs=2, space="PSUM") as scp,
            tc.tile_pool(name="oacc", bufs=1, space="PSUM") as oap,
            tc.tile_pool(name="pj", bufs=2, space="PSUM") as pjp,
            tc.tile_pool(name="pt", bufs=PTBUFS) as ptp,
            tc.tile_pool(name="rc", bufs=4) as rcp,
            tc.tile_pool(name="osb", bufs=4) as oup,
        ):
            xT_sb = pp.tile([128, KC * S], BF, tag="xT", name="xT_sb")
            # halves-major: col = half*1024 + k*128 + d
            wq_sb = pp.tile([128, 2 * 1024], BF, tag="wq", name="wq_sb")
            wk_sb = pp.tile([128, 2 * 1024], BF, tag="wk", name="wk_sb")
            wv_sb = pp.tile([128, 2 * 1024], BF, tag="wv", name="wv_sb")
            wo_sb = pp.tile([128, 2 * D], BF, tag="wo", name="wo_sb")
            qt_sb = pp.tile([128, 2 * S], BF, tag="qt", name="qt_sb")
            kt_sb = pp.tile([128, 2 * S], BF, tag="kt", name="kt_sb")
            v_sb = pp.tile([128, SB * VP], BF, tag="v", name="v_sb")
            onm_sb = pp.tile([128, 32 * 128], BF, tag="onm", name="onm_sb")
            onmT_sb = pp.tile([128, 2 * S], BF, tag="onmT", name="onmT_sb")
            ident = pp.tile([128, 128], BF, tag="ident", name="ident")
            if with_bias:
                bias_sb = pp.tile([1, 3 * DHC], BF, tag="bias", name="bias_sb")
                ones16 = pp.tile([1, 512], BF, tag="ones16", name="ones16")

            # PE p-state warmup: matmul cost is locked at sequencer visit
            # time from the current busy-ramp; a dense burst of dummy matmuls
            # at t=0 ramps the PE to full clock before the (DMA-gated) lead-in
            # projections are visited, and the ramp persists across idle gaps.
            from concourse.masks import make_identity
            make_identity(nc, ident[:, :])
            warm = scp.tile([128, 128], F32, tag="sc", name="warm")
            for _ in range(WARM_N if USE_WARM else 0):
                nc.tensor.matmul(
                    warm[:, :], lhsT=ident[:, :], rhs=ident[:, :],
                    start=True, stop=True,
                )

            # ---- input DMAs.  wq/wk first (gate the lead-in projections);
            # xT streamed in s-quarters, later quarters split between the SP
            # HWDGE queue and the Pool SWDGE queue to beat the per-DMA HWDGE
            # overhead; wv after quarter 0; wo last.
            def load_w(w_sb, w_d, half):
                nc.sync.dma_start(
                    w_sb[:, half * 1024:(half + 1) * 1024],
                    w_d[half * 128:(half + 1) * 128, :],
                )

            def load_xt_q(k, nt, engine):
                engine.dma_start(
                    xT_sb[:, k * S + nt * 512: k * S + (nt + 1) * 512],
                    xT_d[k * 128:(k + 1) * 128, nt * 512:(nt + 1) * 512],
                )

            # The DMA rings serve transfers in readiness order; the lead-in is
            # transfer-bound.  Load only the p0 column halves of wq/wk before
            # the quarter-0 xT chunks (all the lead-in needs), split quarter-0
            # between the SP HWDGE queue (k 0-3) and the Pool SWDGE queue
            # (k 4-7), and defer everything else.
            load_w(wq_sb, wq_d, 0)
            load_w(wk_sb, wk_d, 0)
            if with_bias:
                nc.sync.dma_start(bias_sb[:, :], bias_d[:, :])
            for k in range(4):
                load_xt_q(k, 0, nc.sync)
            for k in range(4, KC):
                load_xt_q(k, 0, nc.gpsimd if XT_SWDGE else nc.sync)
            load_w(wv_sb, wv_d, 0)
            # ones columns of V_aug: preset all of v_sb (V overwrites below)
            nc.gpsimd.memset(v_sb[:, :], 1.0)
            for nt in range(1, 4):
                for k in range(4):
                    load_xt_q(k, nt, nc.sync)
                for k in range(4, KC):
                    load_xt_q(k, nt, nc.gpsimd if XT_SWDGE else nc.sync)
            load_w(wq_sb, wq_d, 1)
            load_w(wk_sb, wk_d, 1)
            load_w(wv_sb, wv_d, 1)
            nc.sync.dma_start(
                wo_sb[:, :].rearrange("r (p d) -> r p d", d=D),
                wo_d[:, :].rearrange("(p r) d -> r p d", r=128),
            )
            from concourse.masks import make_identity
            make_identity(nc, ident[:, :])
            if with_bias:
                nc.vector.memset(ones16[:, :], 1.0)
                bq = bias_sb[0:1, 0:DHC]
                bk = bias_sb[0:1, DHC:2 * DHC]
                bv = bias_sb[0:1, 2 * DHC:3 * DHC]

            # ================= projection emitters =================
            def qk_mm(ps, w_sb, p, nt, k):
                nc.tensor.matmul(
                    ps[:, :],
                    lhsT=w_sb[:, p * 1024 + k * 128: p * 1024 + (k + 1) * 128],
                    rhs=xT_sb[:, k * S + nt * 512: k * S + (nt + 1) * 512],
                    start=(k == 0),
                    stop=(k == KC - 1 and not with_bias),
                )

            def qk_fin(ps, dst, bias, p, nt, on_act=False):
                if with_bias:
                    nc.tensor.matmul(
                        ps[:, :],
                        lhsT=bias[:, p * 128:(p + 1) * 128],
                        rhs=ones16[0:1, :],
                        start=False,
                        stop=True,
                    )
                dslice = dst[:, p * S + nt * 512: p * S + (nt + 1) * 512]
                if on_act:
                    nc.scalar.copy(dslice, ps[:, :])
                else:
                    nc.vector.tensor_copy(dslice, ps[:, :])

            def qk_mm_w(ps, w_sb, p, s0, w, k):
                nc.tensor.matmul(
                    ps[:, :],
                    lhsT=w_sb[:, p * 1024 + k * 128: p * 1024 + (k + 1) * 128],
                    rhs=xT_sb[:, k * S + s0: k * S + s0 + w],
                    start=(k == 0),
                    stop=(k == KC - 1 and not with_bias),
                )

            def qk_fin_w(ps, dst, bias, p, s0, w, on_act=False):
                if with_bias:
                    nc.tensor.matmul(
                        ps[:, :],
                        lhsT=bias[:, p * 128:(p + 1) * 128],
                        rhs=ones16[0:1, 0:w],
                        start=False,
                        stop=True,
                    )
                dslice = dst[:, p * S + s0: p * S + s0 + w]
                if on_act:
                    nc.scalar.copy(dslice, ps[:, :])
                else:
                    nc.vector.tensor_copy(dslice, ps[:, :])

            def v_mm(ps, sb, pair, k):
                # V natural: out[s,dhc] = sum_d x[s,d] wv[d,dhc]; xT chunk stationary
                nc.tensor.matmul(
                    ps[:, :],
                    lhsT=xT_sb[:, k * S + sb * 128: k * S + (sb + 1) * 128],
                    rhs=wv_sb[:, pair * 1024 + k * 128: pair * 1024 + (k + 1) * 128],
                    start=(k == 0),
                    stop=(k == KC - 1 and not with_bias),
                )

            def v_fin(ps, sb, pair):
                if with_bias:
                    nc.tensor.matmul(
                        ps[:, :],
                        lhsT=ones16[0:1, 0:128],
                        rhs=bv[0:1, pair * 128:(pair + 1) * 128],
                        start=False,
                        stop=True,
                    )
                dst = v_sb[
                    :, sb * VP + 2 * pair * (HD + 1): sb * VP + (2 * pair + 2) * (HD + 1)
                ].rearrange("p (h e) -> p h e", e=HD + 1)[:, :, 0:HD]
                src = ps[:, :].rearrange("p (h e) -> p h e", e=HD)
                nc.vector.tensor_copy(dst, src)

            # ================= filler unit list =================
            # Each unit: (deadline_slot, tile_key, emit_fn, cost_ns).
            # Units of one tile must stay consecutive w.r.t. the pj pool ring;
            # the scheduler emits units in list order gated by deadline/budget.
            units = []
            v_emitted = [[False] * SB, [False] * SB]  # [pair][sb]

            def add_qk_tile(dl, dst, w_sb, bias, p, nt):
                state = {}

                def mk(kk):
                    def f():
                        if kk == 0:
                            state["ps"] = pjp.tile(
                                [128, 512], F32, tag="pj", name=f"qk{p}{nt}"
                            )
                        qk_mm(state["ps"], w_sb, p, nt, kk)
                        qk_mm(state["ps"], w_sb, p, nt, kk + 1)
                        if kk == KC - 2:
                            qk_fin(state["ps"], dst, bias, p, nt)
                    return f

                for kk in range(0, KC, 2):
                    units.append([dl, mk(kk), 430])

            def add_v_tile(dl, sb, pair):
                state = {}

                def mk(kk):
                    def f():
                        if kk == 0:
                            state["ps"] = pjp.tile(
                                [128, 128], F32, tag="pj", name=f"v{sb}_{pair}"
                            )
                        for k2 in range(kk, kk + 4):
                            v_mm(state["ps"], sb, pair, k2)
                        if kk == KC - 4:
                            v_fin(state["ps"], sb, pair)
                            v_emitted[pair][sb] = True
                    return f

                for kk in range(0, KC, 4):
                    units.append([dl, mk(kk), 220, ("v", pair, sb)])

            bq_ = bk_ = None
            if with_bias:
                bq_, bk_ = bq, bk

            def add_ktrest():
                state = {}

                def mk(kk):
                    def f():
                        if kk == 0:
                            state["ps"] = pjp.tile(
                                [128, 256], F32, tag="pj", name="kt_rest"
                            )
                        for k2 in range(kk, kk + 4):
                            qk_mm_w(state["ps"], wk_sb, 0, 256, 256, k2)
                        if kk == KC - 4:
                            qk_fin_w(state["ps"], kt_sb, bk_, 0, 256, 256)
                    return f

                for kk in range(0, KC, 4):
                    units.append([-1, mk(kk), 440])

            add_ktrest()
            # deadlines in body-slot units (slot = (hp*4+qw)*16 + c)
            add_qk_tile(2, kt_sb, wk_sb, bk_, 0, 1)
            add_qk_tile(6, kt_sb, wk_sb, bk_, 0, 2)
            add_qk_tile(10, kt_sb, wk_sb, bk_, 0, 3)
            add_qk_tile(13, qt_sb, wq_sb, bq_, 0, 1)
            v0off = _K('V0OFF', 6)
            for sb in range(SB):
                add_v_tile(max(_K('V0MIN', 2), v0off + sb), sb, 0)
            add_qk_tile(28, qt_sb, wq_sb, bq_, 0, 2)
            add_qk_tile(44, qt_sb, wq_sb, bq_, 0, 3)
            add_qk_tile(54, kt_sb, wk_sb, bk_, 1, 0)
            add_qk_tile(58, qt_sb, wq_sb, bq_, 1, 0)
            add_qk_tile(62, kt_sb, wk_sb, bk_, 1, 1)
            add_qk_tile(68, kt_sb, wk_sb, bk_, 1, 2)
            add_qk_tile(72, kt_sb, wk_sb, bk_, 1, 3)
            add_qk_tile(76, qt_sb, wq_sb, bq_, 1, 1)
            for sb in range(SB):
                add_v_tile(_K('V1OFF', 78) + sb, sb, 1)
            add_qk_tile(92, qt_sb, wq_sb, bq_, 1, 2)
            add_qk_tile(106, qt_sb, wq_sb, bq_, 1, 3)

            # outproj: gated on onmT of BOTH pairs for the tile's q window.
            op_ready = [[False, False] for _ in range(NQW)]  # [qw][hp]

            def add_outproj(sb):
                qw = sb // 4
                state = {}

                def mkmm(n):
                    def f():
                        pool, tg = (
                            (scp, "sc") if sb >= 12 and sb % 2 == 1 else (pjp, "pj")
                        )
                        ps = pool.tile([128, 512], F32, tag=tg, name=f"op{sb}_{n}")
                        state[n] = ps
                        if n == 0:
                            state["ot"] = oup.tile(
                                [128, 1024], BF, tag="osb", name=f"ot{sb}"
                            )
                        for p in range(2):
                            nc.tensor.matmul(
                                ps[:, :],
                                lhsT=onmT_sb[:, p * S + sb * 128:p * S + (sb + 1) * 128],
                                rhs=wo_sb[:, p * D + n * 512: p * D + (n + 1) * 512],
                                start=(p == 0),
                                stop=(p == 1),
                            )
                        dst = state["ot"][:, n * 512:(n + 1) * 512]
                        if sb >= 12 and n == 1:
                            # tail tiles: ACT is idle after the final exp
                            nc.scalar.copy(dst, ps[:, :])
                        else:
                            nc.vector.tensor_copy(dst, ps[:, :])
                        if n == 1:
                            nc.sync.dma_start(
                                out_d[sb * 128:(sb + 1) * 128, :], state["ot"][:, :]
                            )
                    return f

                dl = 64 + 16 * qw + (24 if OLDSCHED else (18 if qw >= 2 else 24))
                for n in range(2):
                    units.append([dl, mkmm(n), 520, ("op", qw)])

            for sb in range(SB):
                add_outproj(sb)

            units.sort(key=lambda u: u[0])

            # ================= attention =================
            def emit_pv(hp, qw, c, pt_t, oA, oB):
                # PSUM `start` marks the whole 2KB bank pending-zero, so only
                # the FIRST strip written into each oacc bank may carry it;
                # the other strips' first write then overwrites (pending) and
                # later chunks accumulate.
                for i in range(2):
                    for qb in range(4):
                        oacc = oA if qb < 2 else oB
                        loc = (qb % 2) * 2 + i
                        nc.tensor.matmul(
                            oacc[:, loc * 65: loc * 65 + 65],
                            lhsT=pt_t[:, i * 512 + qb * 128: i * 512 + (qb + 1) * 128],
                            rhs=v_sb[
                                :, c * VP + (2 * hp + i) * 65:
                                c * VP + (2 * hp + i) * 65 + 65
                            ],
                            start=(c == 0 and loc == 0),
                            stop=(c == SB - 1),
                            skip_group_check=True,
                        )

            def emit_drain(hp, qw, oA, oB):
                last = (hp, qw) == (1, NQW - 1)
                rcs = {}
                for qb in range(4):
                    oacc = oA if qb < 2 else oB
                    t = 0 if qb < 2 else 1
                    if t not in rcs:
                        rc = rcp.tile([128, 4], F32, tag="rc", name=f"rc{hp}{qw}{t}")
                        nc.vector.reciprocal_approx_fast(
                            out=rc[:, :].rearrange("p (s e) -> p s e", e=1),
                            in_=oacc[:, :].rearrange("p (s e) -> p s e", e=65)[
                                :, :, 64:65
                            ],
                        )
                        rcs[t] = rc
                    rc = rcs[t]
                    blk = ((hp * 4 + qw) * 4 + qb) * 128
                    for i in range(2):
                        loc = (qb % 2) * 2 + i
                        dst = onm_sb[:, blk + i * 64: blk + (i + 1) * 64]
                        src = oacc[:, loc * 65: loc * 65 + 64]
                        if last and i == 1 and USE_ACTMUL:
                            # ACT is idle after the final exp: split the
                            # normalize multiplies across both engines
                            nc.scalar.mul(dst, src, rc[:, loc:loc + 1])
                        else:
                            nc.vector.tensor_scalar_mul(dst, src, rc[:, loc:loc + 1])
                    dstT = onmT_sb[:, hp * S + qw * 512 + qb * 128:
                                   hp * S + qw * 512 + (qb + 1) * 128]
                    if last and USE_PETP:
                        # PE + ACT/DVE are idle after the final exp and the
                        # HWDGE queue is needed for the output DMAs: transpose
                        # on the PE instead of the DMA xbar.
                        tp = scp.tile([128, 128], BF, tag="sc", name=f"tp{qb}")
                        nc.tensor.transpose(tp[:, :], onm_sb[:, blk:blk + 128],
                                            ident[:, :])
                        if qb % 2 == 0:
                            nc.scalar.copy(dstT, tp[:, :])
                        else:
                            nc.vector.tensor_copy(dstT, tp[:, :])
                    else:
                        nc.sync.dma_start(dstT, onm_sb[:, blk:blk + 128],
                                          transpose=True)
                op_ready[qw][hp] = True

            # ---- lead-in: QT/KT (pair 0, window 0) pipelined against the
            # arriving xT quarter-0 chunks; fins on ACT (idle before attention)
            lead_qt = pjp.tile([128, 512], F32, tag="pj", name="lead_qt")
            lead_kt = pjp.tile([128, 256], F32, tag="pj", name="lead_kt")
            # consume xT chunks in their expected arrival order (Pool SWDGE
            # k4-7 descriptors generate faster than the SP HWDGE chain).  The
            # KT lead tile covers only kp chunks 0-1 (256 cols) so the first
            # exp fires as soon as the last quarter lands; chunks 2-3 follow
            # as the first filler unit.
            lead_korder = [4, 0, 5, 1, 6, 2, 7, 3]
            for ki, k in enumerate(lead_korder):
                st, sp = (ki == 0), (ki == KC - 1 and not with_bias)
                nc.tensor.matmul(
                    lead_qt[:, :],
                    lhsT=wq_sb[:, k * 128:(k + 1) * 128],
                    rhs=xT_sb[:, k * S: k * S + 512],
                    start=st, stop=sp,
                )
                nc.tensor.matmul(
                    lead_kt[:, :],
                    lhsT=wk_sb[:, k * 128:(k + 1) * 128],
                    rhs=xT_sb[:, k * S: k * S + 256],
                    start=st, stop=sp,
                )
            # QT fin on ACT, KT fin on DVE: both engines idle here, and the
            # first score matmul needs both done.
            qk_fin_w(lead_qt, qt_sb, bq_, 0, 0, 512, on_act=True)
            qk_fin_w(lead_kt, kt_sb, bk_, 0, 0, 256, on_act=False)

            # ---- main loop
            ui = 0           # next filler unit
            pending = []     # FIFO of (hp, qw, c, pt_tile)
            oacc_cur = {}    # (hp,qw) -> (oA, oB)
            drained = set()

            def get_oacc(hp, qw):
                if (hp, qw) not in oacc_cur:
                    oacc_cur[(hp, qw)] = (
                        oap.tile([128, 260], F32, tag="oA", name=f"oA{hp}{qw}"),
                        oap.tile([128, 260], F32, tag="oB", name=f"oB{hp}{qw}"),
                    )
                return oacc_cur[(hp, qw)]

            def pv_head_ready():
                hp_, qw_, c_, _ = pending[0]
                return v_emitted[hp_][c_]

            def force_v(pair, sb):
                """Emit all remaining units of V tile (pair, sb) immediately.
                Out-of-order vs the deadline list is fine for the pj ring as
                long as tile-internal unit order is preserved."""
                j = ui
                while not v_emitted[pair][sb]:
                    assert j < len(units), f"no units left for V({pair},{sb})"
                    u = units[j]
                    if len(u) > 3 and u[3] == ("v", pair, sb):
                        u[1]()
                        units.pop(j)
                    else:
                        j += 1

            def pop_pv(maxn):
                n = 0
                while pending and n < maxn:
                    if not pv_head_ready():
                        break
                    hp_, qw_, c_, pt_ = pending.pop(0)
                    oA, oB = get_oacc(hp_, qw_)
                    emit_pv(hp_, qw_, c_, pt_, oA, oB)
                    n += 1
                    if c_ == SB - 1:
                        emit_drain(hp_, qw_, oA, oB)
                        drained.add((hp_, qw_))
                        del oacc_cur[(hp_, qw_)]

            budget_rate = float(_K('BRATE', 500))
            budget = -2000.0

            slots = [
                (hp, qw, c)
                for hp in range(2) for qw in range(NQW) for c in range(SB)
            ]

            def emit_qk(hp, qw, c, slot):
                # scores for chunk c (both heads packed)
                sc = scp.tile([128, 1024], F32, tag="sc", name=f"s{slot}")
                for i in range(2):
                    nc.tensor.matmul(
                        sc[:, 512 * i:512 * (i + 1)],
                        lhsT=kt_sb[
                            64 * i:64 * (i + 1),
                            hp * S + c * 128: hp * S + (c + 1) * 128,
                        ],
                        rhs=qt_sb[
                            64 * i:64 * (i + 1),
                            hp * S + qw * 512: hp * S + (qw + 1) * 512,
                        ],
                        start=True,
                        stop=True,
                    )
                return sc

            # QK runs one slot ahead of the exp stream (sc is double-
            # buffered), so deadline-forced fillers sit BEHIND the next
            # slot's QK in the PE queue and can never stall the exp cadence
            # by more than one slot of excess.
            sc_cur = None if OLDSCHED else emit_qk(*slots[0], 0)
            for slot, (hp, qw, c) in enumerate(slots):
                if OLDSCHED:
                    sc_cur = emit_qk(hp, qw, c, slot)
                # keep the pt ring from stalling the exp stream
                if len(pending) >= _K('PTCAP', 14):
                    hp_, qw_, c_, _ = pending[0]
                    if not v_emitted[hp_][c_]:
                        force_v(hp_, c_)
                    pop_pv(4)
                pt_t = ptp.tile([128, 1024], BF, tag="pt", name=f"p{slot}")
                nc.scalar.activation(pt_t[:, :], sc_cur[:, :], EXP, scale=0.125)
                pending.append((hp, qw, c, pt_t))
                if not OLDSCHED and slot + 1 < len(slots):
                    sc_cur = emit_qk(*slots[slot + 1], slot + 1)
                if (not OLDSCHED) and hp == 1 and qw >= NQW - 2:
                    # last windows: drain PV eagerly so the post-exp tail
                    # (drain -> transpose -> outproj) starts early
                    pop_pv(3)
                else:
                    pop_pv(2 if len(pending) > _K('POPTH', 6) else 1)
                # fillers by deadline + budget
                budget = min(budget + budget_rate, 900.0)
                while ui < len(units):
                    u = units[ui]
                    if u[0] > slot and budget < u[2]:
                        break
                    if len(u) > 3 and u[3][0] == "op":
                        qw_ = u[3][1]
                        if not (op_ready[qw_][0] and op_ready[qw_][1]):
                            break
                    u[1]()
                    budget -= u[2]
                    ui += 1

            # ---- tail
            while pending:
                if not pv_head_ready():
                    hp_, qw_, c_, _ = pending[0]
                    force_v(hp_, c_)
                pop_pv(99)
            while ui < len(units):
                u = units[ui]
                if len(u) > 3 and u[3][0] == "op":
                    qw_ = u[3][1]
                    assert op_ready[qw_][0] and op_ready[qw_][1]
                u[1]()
                ui += 1

            if debug:
                nc.sync.dma_start(dbg["qt"][:, :], qt_sb[:, :])
                nc.sync.dma_start(dbg["kt"][:, :], kt_sb[:, :])
                nc.sync.dma_start(dbg["v"][:, :], v_sb[:, :])
                nc.sync.dma_start(dbg["onm"][:, :], onm_sb[:, :])
                nc.sync.dma_start(dbg["onmT"][:, :], onmT_sb[:, :])

    nc.compile()
    return nc


def _get_nc(with_bias=False):
    if with_bias not in _nc_cache:
        _nc_cache[with_bias] = _build_bass(with_bias=with_bias)
    return _nc_cache[with_bias]


def _prepare_in_maps(x, wq, bq, wk, bk, wv, bv, wo, with_bias):
    import ml_dtypes

    bf16 = ml_dtypes.bfloat16
    x = np.asarray(x, np.float32)
    wq, bq = np.asarray(wq, np.float32), np.asarray(bq, np.float32)
    wk, bk = np.asarray(wk, np.float32), np.asarray(bk, np.float32)
    wv, bv = np.asarray(wv, np.float32), np.asarray(bv, np.float32)
    wo = np.asarray(wo, np.float32)

    xT = [np.ascontiguousarray(x[b].T).astype(bf16) for b in range(B)]

    def swz(w):
        # [k*128+p, h*128+c] -> [h*128+p, k*128+c]: each output-column half
        # becomes one contiguous [128, 1024] block (single fast DMA).
        return np.ascontiguousarray(
            w.reshape(KC, 128, 2, 128).transpose(2, 1, 0, 3).reshape(DHC, D)
        ).astype(bf16)

    in_maps = []
    for c in range(NC):
        b, j = divmod(c, HPC)
        cs = slice(DHC * j, DHC * (j + 1))
        m = {
            "xT": xT[b],
            "wq_c": swz(wq[:, cs]),
            "wk_c": swz(wk[:, cs]),
            "wv_c": swz(wv[:, cs]),
            "wo_c": np.ascontiguousarray(wo[cs, :]).astype(bf16),
        }
        if with_bias:
            bias3 = np.concatenate([bq[cs], bk[cs], bv[cs]]).reshape(1, 3 * DHC)
            m["bias3"] = np.ascontiguousarray(bias3.astype(bf16))
        in_maps.append(m)
    return in_maps


def kernel(x, wq, bq, wk, bk, wv, bv, wo, bo):
    from concourse import bass_utils

    with_bias = bool(
        np.any(np.asarray(bq)) or np.any(np.asarray(bk)) or np.any(np.asarray(bv))
    )
    in_maps = _prepare_in_maps(x, wq, bq, wk, bk, wv, bv, wo, with_bias)
    res = bass_utils.run_bass_kernel_spmd(
        nc=_get_nc(with_bias), in_maps=in_maps, core_ids=list(range(NC))
    )
    bo = np.asarray(bo, np.float32)
    out = np.empty((B, S, D), np.float32)
    for b in range(B):
        acc = np.asarray(res.results[HPC * b]["out"], np.float32)
        for j in range(1, HPC):
            acc = acc + np.asarray(res.results[HPC * b + j]["out"], np.float32)
        out[b] = acc + bo
    return out


# revision 37
# speedup vs baseline: 1.2293x; 1.0048x over previous
"""Trainium2 Bass kernel for nn_Attention_80384607912675.

Multi-head attention (B=2, S=2048, D=1024, H=16, HD=64), fp32 reference,
bf16 on-chip compute.

Sharding (8 cores): data-parallel over batch (2) x tensor-parallel over heads
(4 head groups of 4 heads).  Core c handles batch c//4, heads [4*(c%4), 4*(c%4)+4).
wq/wk/wv split column-wise, wo split row-wise; the wo partials (4 per batch)
and bias bo are reduced on the host.

Per-core structure (head pairs hp in {0,1}, 512-wide q windows qw, kp chunks c):
  QT/KT  = (x @ wq/k)^T head-major [128, 2048] per pair     (wq chunk stationary)
  V      = x @ wv natural [s, 4*(64+1)] per s-chunk          (xT chunk stationary;
           ones column per head folds the softmax row-sum into PV)
  S^T[kp, q] = K_h^T (x) Q_h            (K=64, kt chunk stationary, N=512)
  P^T    = exp(S^T / 8)  -> bf16        (one ScalarE instr per (hp,qw,c))
  O_nat[q, h*65] += P^T-slice (x) V_aug (P STATIONARY, N=65: 2x fewer PE
                                         cycles than V-stationary)
  normalize: 1/rowsum via DVE reciprocal_approx_fast, per-partition
             tensor_scalar_mul -> onm natural bf16
  onm^T via DMA-transpose engine (no PE cycles)
  out[q, :] = onm^T-slices @ wo (both pairs accumulated) -> single output

The exp stream on ScalarE (~133us) and the PE matmul stream (~137us) are both
near-critical; all projection/outproj work is interleaved into the attention
loop by a deadline-driven scheduler so neither engine starves.
"""

import numpy as np
import os
_K = lambda n, d: type(d)(os.environ.get(n, d))

B, S, D, H = 2, 2048, 1024, 16
HD = D // H          # 64
HPC = 4              # heads per core
DHC = HPC * HD       # 256 head dims per core
KC = D // 128        # 8 contraction chunks
SB = S // 128        # 16 s blocks / kp chunks
VP = HPC * (HD + 1)  # 260: V storage pitch per s-chunk (ones col per head)
NC = 8               # cores
NQW = 4              # 512-wide q windows per head pair

_nc_cache = {}


def _build_bass(with_bias=False, debug=False):
    PTBUFS = _K('PTBUFS', 16)
    import concourse.mybir as mybir
    import concourse.tile as tile
    from concourse import bacc

    BF = mybir.dt.bfloat16
    F32 = mybir.dt.float32
    EXP = mybir.ActivationFunctionType.Exp

    nc = bacc.Bacc("TRN2")

    xT_d = nc.dram_tensor("xT", [D, S], BF, kind="ExternalInput")
    # wq/wk/wv arrive host-swizzled as [2 halves * 128 cols, 8 chunks * 128 d]
    # so each half loads as one contiguous [128, 1024] DMA (no small-run 2x
    # descriptor penalty).
    wq_d = nc.dram_tensor("wq_c", [DHC, D], BF, kind="ExternalInput")
    wk_d = nc.dram_tensor("wk_c", [DHC, D], BF, kind="ExternalInput")
    wv_d = nc.dram_tensor("wv_c", [DHC, D], BF, kind="ExternalInput")
    wo_d = nc.dram_tensor("wo_c", [DHC, D], BF, kind="ExternalInput")
    if with_bias:
        bias_d = nc.dram_tensor("bias3", [1, 3 * DHC], BF, kind="ExternalInput")
    out_d = nc.dram_tensor("out", [S, D], BF, kind="ExternalOutput")
    if debug:
        dbg = {
            "qt": nc.dram_tensor("dbg_qt", [128, 2 * S], BF, kind="ExternalOutput"),
            "kt": nc.dram_tensor("dbg_kt", [128, 2 * S], BF, kind="ExternalOutput"),
            "v": nc.dram_tensor("dbg_v", [128, SB * VP], BF, kind="ExternalOutput"),
            "onm": nc.dram_tensor("dbg_onm", [128, 32 * 128], BF, kind="ExternalOutput"),
            "onmT": nc.dram_tensor("dbg_onmT", [128, 2 * S], BF, kind="ExternalOutput"),
        }

    with tile.TileContext(nc) as tc:
        with (
            tc.tile_pool(name="persist", bufs=1) as pp,
            tc.tile_pool(name="sc", bufs=2, space="PSUM") as scp,
            tc.tile_pool(name="oacc", bufs=1, space="PSUM") as oap,
            tc.tile_pool(name="pj", bufs=2, space="PSUM") as pjp,
            tc.tile_pool(name="pt", bufs=PTBUFS) as ptp,
            tc.tile_pool(name="rc", bufs=4) as rcp,
            tc.tile_pool(name="osb", bufs=4) as oup,
        ):
            xT_sb = pp.tile([128, KC * S], BF, tag="xT", name="xT_sb")
            # halves-major: col = half*1024 + k*128 + d
            wq_sb = pp.tile([128, 2 * 1024], BF, tag="wq", name="wq_sb")
            wk_sb = pp.tile([128, 2 * 1024], BF, tag="wk", name="wk_sb")
            wv_sb = pp.tile([128, 2 * 1024], BF, tag="wv", name="wv_sb")
            wo_sb = pp.tile([128, 2 * D], BF, tag="wo", name="wo_sb")
            qt_sb = pp.tile([128, 2 * S], BF, tag="qt", name="qt_sb")
            kt_sb = pp.tile([128, 2 * S], BF, tag="kt", name="kt_sb")
            v_sb = pp.tile([128, SB * VP], BF, tag="v", name="v_sb")
            onm_sb = pp.tile([128, 32 * 128], BF, tag="onm", name="onm_sb")
            onmT_sb = pp.tile([128, 2 * S], BF, tag="onmT", name="onmT_sb")
            ident = pp.tile([128, 128], BF, tag="ident", name="ident")
            if with_bias:
                bias_sb = pp.tile([1, 3 * DHC], BF, tag="bias", name="bias_sb")
                ones16 = pp.tile([1, 512], BF, tag="ones16", name="ones16")

            # PE p-state warmup: matmul cost is locked at sequencer visit
            # time from the current busy-ramp; a dense burst of dummy matmuls
            # at t=0 ramps the PE to full clock before the (DMA-gated) lead-in
            # projections are visited, and the ramp persists across idle gaps.
            from concourse.masks import make_identity
            make_identity(nc, ident[:, :])
            warm = scp.tile([128, 128], F32, tag="sc", name="warm")
            for _ in range(WARM_N if USE_WARM else 0):
                nc.tensor.matmul(
                    warm[:, :], lhsT=ident[:, :], rhs=ident[:, :],
                    start=True, stop=True,
                )

            # ---- input DMAs.  wq/wk first (gate the lead-in projections);
            # xT streamed in s-quarters, later quarters split between the SP
            # HWDGE queue and the Pool SWDGE queue to beat the per-DMA HWDGE
            # overhead; wv after quarter 0; wo last.
            def load_w(w_sb, w_d, half):
                nc.sync.dma_start(
                    w_sb[:, half * 1024:(half + 1) * 1024],
                    w_d[half * 128:(half + 1) * 128, :],
                )

            def load_xt_q(k, nt, engine):
                engine.dma_start(
                    xT_sb[:, k * S + nt * 512: k * S + (nt + 1) * 512],
                    xT_d[k * 128:(k + 1) * 128, nt * 512:(nt + 1) * 512],
                )

            # The DMA rings serve transfers in readiness order; the lead-in is
            # transfer-bound.  Load only the p0 column halves of wq/wk before
            # the quarter-0 xT chunks (all the lead-in needs), split quarter-0
            # between the SP HWDGE queue (k 0-3) and the Pool SWDGE queue
            # (k 4-7), and defer everything else.
            load_w(wq_sb, wq_d, 0)
            load_w(wk_sb, wk_d, 0)
            if with_bias:
                nc.sync.dma_start(bias_sb[:, :], bias_d[:, :])
            for k in range(4):
                load_xt_q(k, 0, nc.sync)
            for k in range(4, KC):
                load_xt_q(k, 0, nc.gpsimd if XT_SWDGE else nc.sync)
            load_w(wv_sb, wv_d, 0)
            # ones columns of V_aug: preset all of v_sb (V overwrites below)
            nc.gpsimd.memset(v_sb[:, :], 1.0)
            for nt in range(1, 4):
                for k in range(4):
                    load_xt_q(k, nt, nc.sync)
                for k in range(4, KC):
                    load_xt_q(k, nt, nc.gpsimd if XT_SWDGE else nc.sync)
            load_w(wq_sb, wq_d, 1)
            load_w(wk_sb, wk_d, 1)
            load_w(wv_sb, wv_d, 1)
            nc.sync.dma_start(
                wo_sb[:, :].rearrange("r (p d) -> r p d", d=D),
                wo_d[:, :].rearrange("(p r) d -> r p d", r=128),
            )
            from concourse.masks import make_identity
            make_identity(nc, ident[:, :])
            if with_bias:
                nc.vector.memset(ones16[:, :], 1.0)
                bq = bias_sb[0:1, 0:DHC]
                bk = bias_sb[0:1, DHC:2 * DHC]
                bv = bias_sb[0:1, 2 * DHC:3 * DHC]

            # ================= projection emitters =================
            def qk_mm(ps, w_sb, p, nt, k):
                nc.tensor.matmul(
                    ps[:, :],
                    lhsT=w_sb[:, p * 1024 + k * 128: p * 1024 + (k + 1) * 128],
                    rhs=xT_sb[:, k * S + nt * 512: k * S + (nt + 1) * 512],
                    start=(k == 0),
                    stop=(k == KC - 1 and not with_bias),
                )

            def qk_fin(ps, dst, bias, p, nt, on_act=False):
                if with_bias:
                    nc.tensor.matmul(
                        ps[:, :],
                        lhsT=bias[:, p * 128:(p + 1) * 128],
                        rhs=ones16[0:1, :],
                        start=False,
                        stop=True,
                    )
                dslice = dst[:, p * S + nt * 512: p * S + (nt + 1) * 512]
                if on_act:
                    nc.scalar.copy(dslice, ps[:, :])
                else:
                    nc.vector.tensor_copy(dslice, ps[:, :])

            def qk_mm_w(ps, w_sb, p, s0, w, k):
                nc.tensor.matmul(
                    ps[:, :],
                    lhsT=w_sb[:, p * 1024 + k * 128: p * 1024 + (k + 1) * 128],
                    rhs=xT_sb[:, k * S + s0: k * S + s0 + w],
                    start=(k == 0),
                    stop=(k == KC - 1 and not with_bias),
                )

            def qk_fin_w(ps, dst, bias, p, s0, w, on_act=False):
                if with_bias:
                    nc.tensor.matmul(
                        ps[:, :],
                        lhsT=bias[:, p * 128:(p + 1) * 128],
                        rhs=ones16[0:1, 0:w],
                        start=False,
                        stop=True,
                    )
                dslice = dst[:, p * S + s0: p * S + s0 + w]
                if on_act:
                    nc.scalar.copy(dslice, ps[:, :])
                else:
                    nc.vector.tensor_copy(dslice, ps[:, :])

            def v_mm(ps, sb, pair, k):
                # V natural: out[s,dhc] = sum_d x[s,d] wv[d,dhc]; xT chunk stationary
                nc.tensor.matmul(
                    ps[:, :],
                    lhsT=xT_sb[:, k * S + sb * 128: k * S + (sb + 1) * 128],
                    rhs=wv_sb[:, pair * 1024 + k * 128: pair * 1024 + (k + 1) * 128],
                    start=(k == 0),
                    stop=(k == KC - 1 and not with_bias),
                )

            def v_fin(ps, sb, pair):
                if with_bias:
                    nc.tensor.matmul(
                        ps[:, :],
                        lhsT=ones16[0:1, 0:128],
                        rhs=bv[0:1, pair * 128:(pair + 1) * 128],
                        start=False,
                        stop=True,
                    )
                dst = v_sb[
                    :, sb * VP + 2 * pair * (HD + 1): sb * VP + (2 * pair + 2) * (HD + 1)
                ].rearrange("p (h e) -> p h e", e=HD + 1)[:, :, 0:HD]
                src = ps[:, :].rearrange("p (h e) -> p h e", e=HD)
                nc.vector.tensor_copy(dst, src)

            # ================= filler unit list =================
            # Each unit: (deadline_slot, tile_key, emit_fn, cost_ns).
            # Units of one tile must stay consecutive w.r.t. the pj pool ring;
            # the scheduler emits units in list order gated by deadline/budget.
            units = []
            v_emitted = [[False] * SB, [False] * SB]  # [pair][sb]

            def add_qk_tile(dl, dst, w_sb, bias, p, nt):
                state = {}

                def mk(kk):
                    def f():
                        if kk == 0:
                            state["ps"] = pjp.tile(
                                [128, 512], F32, tag="pj", name=f"qk{p}{nt}"
                            )
                        qk_mm(state["ps"], w_sb, p, nt, kk)
                        qk_mm(state["ps"], w_sb, p, nt, kk + 1)
                        if kk == KC - 2:
                            qk_fin(state["ps"], dst, bias, p, nt)
                    return f

                for kk in range(0, KC, 2):
                    units.append([dl, mk(kk), 430])

            def add_v_tile(dl, sb, pair):
                state = {}

                def mk(kk):
                    def f():
                        if kk == 0:
                            state["ps"] = pjp.tile(
                                [128, 128], F32, tag="pj", name=f"v{sb}_{pair}"
                            )
                        for k2 in range(kk, kk + 4):
                            v_mm(state["ps"], sb, pair, k2)
                        if kk == KC - 4:
                            v_fin(state["ps"], sb, pair)
                            v_emitted[pair][sb] = True
                    return f

                for kk in range(0, KC, 4):
                    units.append([dl, mk(kk), 220, ("v", pair, sb)])

            bq_ = bk_ = None
            if with_bias:
                bq_, bk_ = bq, bk

            def add_ktrest():
                state = {}

                def mk(kk):
                    def f():
                        if kk == 0:
                            state["ps"] = pjp.tile(
                                [128, 256], F32, tag="pj", name="kt_rest"
                            )
                        for k2 in range(kk, kk + 4):
                            qk_mm_w(state["ps"], wk_sb, 0, 256, 256, k2)
                        if kk == KC - 4:
                            qk_fin_w(state["ps"], kt_sb, bk_, 0, 256, 256)
                    return f

                for kk in range(0, KC, 4):
                    units.append([-1, mk(kk), 440])

            add_ktrest()
            # deadlines in body-slot units (slot = (hp*4+qw)*16 + c)
            add_qk_tile(2, kt_sb, wk_sb, bk_, 0, 1)
            add_qk_tile(6, kt_sb, wk_sb, bk_, 0, 2)
            add_qk_tile(10, kt_sb, wk_sb, bk_, 0, 3)
            add_qk_tile(13, qt_sb, wq_sb, bq_, 0, 1)
            v0off = _K('V0OFF', 4)
            for sb in range(SB):
                add_v_tile(max(_K('V0MIN', 1), v0off + sb), sb, 0)
            add_qk_tile(28, qt_sb, wq_sb, bq_, 0, 2)
            add_qk_tile(44, qt_sb, wq_sb, bq_, 0, 3)
            add_qk_tile(54, kt_sb, wk_sb, bk_, 1, 0)
            add_qk_tile(58, qt_sb, wq_sb, bq_, 1, 0)
            add_qk_tile(62, kt_sb, wk_sb, bk_, 1, 1)
            add_qk_tile(68, kt_sb, wk_sb, bk_, 1, 2)
            add_qk_tile(72, kt_sb, wk_sb, bk_, 1, 3)
            add_qk_tile(76, qt_sb, wq_sb, bq_, 1, 1)
            for sb in range(SB):
                add_v_tile(_K('V1OFF', 78) + sb, sb, 1)
            add_qk_tile(92, qt_sb, wq_sb, bq_, 1, 2)
            add_qk_tile(106, qt_sb, wq_sb, bq_, 1, 3)

            # outproj: gated on onmT of BOTH pairs for the tile's q window.
            op_ready = [[False, False] for _ in range(NQW)]  # [qw][hp]

            def add_outproj(sb):
                qw = sb // 4
                state = {}

                def mkmm(n):
                    def f():
                        pool, tg = (
                            (scp, "sc") if sb >= 12 and sb % 2 == 1 else (pjp, "pj")
                        )
                        ps = pool.tile([128, 512], F32, tag=tg, name=f"op{sb}_{n}")
                        state[n] = ps
                        if n == 0:
                            state["ot"] = oup.tile(
                                [128, 1024], BF, tag="osb", name=f"ot{sb}"
                            )
                        for p in range(2):
                            nc.tensor.matmul(
                                ps[:, :],
                                lhsT=onmT_sb[:, p * S + sb * 128:p * S + (sb + 1) * 128],
                                rhs=wo_sb[:, p * D + n * 512: p * D + (n + 1) * 512],
                                start=(p == 0),
                                stop=(p == 1),
                            )
                        dst = state["ot"][:, n * 512:(n + 1) * 512]
                        if sb >= 12 and n == 1:
                            # tail tiles: ACT is idle after the final exp
                            nc.scalar.copy(dst, ps[:, :])
                        else:
                            nc.vector.tensor_copy(dst, ps[:, :])
                        if n == 1:
                            nc.sync.dma_start(
                                out_d[sb * 128:(sb + 1) * 128, :], state["ot"][:, :]
                            )
                    return f

                dl = 64 + 16 * qw + (24 if OLDSCHED else (18 if qw >= 2 else 24))
                for n in range(2):
                    units.append([dl, mkmm(n), 520, ("op", qw)])

            for sb in range(SB):
                add_outproj(sb)

            units.sort(key=lambda u: u[0])

            # ================= attention =================
            def emit_pv(hp, qw, c, pt_t, oA, oB):
                # PSUM `start` marks the whole 2KB bank pending-zero, so only
                # the FIRST strip written into each oacc bank may carry it;
                # the other strips' first write then overwrites (pending) and
                # later chunks accumulate.
                for i in range(2):
                    for qb in range(4):
                        oacc = oA if qb < 2 else oB
                        loc = (qb % 2) * 2 + i
                        nc.tensor.matmul(
                            oacc[:, loc * 65: loc * 65 + 65],
                            lhsT=pt_t[:, i * 512 + qb * 128: i * 512 + (qb + 1) * 128],
                            rhs=v_sb[
                                :, c * VP + (2 * hp + i) * 65:
                                c * VP + (2 * hp + i) * 65 + 65
                            ],
                            start=(c == 0 and loc == 0),
                            stop=(c == SB - 1),
                            skip_group_check=True,
                        )

            def emit_drain(hp, qw, oA, oB):
                last = (hp, qw) == (1, NQW - 1)
                rcs = {}
                for qb in range(4):
                    oacc = oA if qb < 2 else oB
                    t = 0 if qb < 2 else 1
                    if t not in rcs:
                        rc = rcp.tile([128, 4], F32, tag="rc", name=f"rc{hp}{qw}{t}")
                        nc.vector.reciprocal_approx_fast(
                            out=rc[:, :].rearrange("p (s e) -> p s e", e=1),
                            in_=oacc[:, :].rearrange("p (s e) -> p s e", e=65)[
                                :, :, 64:65
                            ],
                        )
                        rcs[t] = rc
                    rc = rcs[t]
                    blk = ((hp * 4 + qw) * 4 + qb) * 128
                    for i in range(2):
                        loc = (qb % 2) * 2 + i
                        dst = onm_sb[:, blk + i * 64: blk + (i + 1) * 64]
                        src = oacc[:, loc * 65: loc * 65 + 64]
                        if last and i == 1 and USE_ACTMUL:
                            # ACT is idle after the final exp: split the
                            # normalize multiplies across both engines
                            nc.scalar.mul(dst, src, rc[:, loc:loc + 1])
                        else:
                            nc.vector.tensor_scalar_mul(dst, src, rc[:, loc:loc + 1])
                    dstT = onmT_sb[:, hp * S + qw * 512 + qb * 128:
                                   hp * S + qw * 512 + (qb + 1) * 128]
                    if last and USE_PETP:
                        # PE + ACT/DVE are idle after the final exp and the
                        # HWDGE queue is needed for the output DMAs: transpose
                        # on the PE instead of the DMA xbar.
                        tp = scp.tile([128, 128], BF, tag="sc", name=f"tp{qb}")
                        nc.tensor.transpose(tp[:, :], onm_sb[:, blk:blk + 128],
                                            ident[:, :])
                        if qb % 2 == 0:
                            nc.scalar.copy(dstT, tp[:, :])
                        else:
                            nc.vector.tensor_copy(dstT, tp[:, :])
                    else:
                        nc.sync.dma_start(dstT, onm_sb[:, blk:blk + 128],
                                          transpose=True)
                op_ready[qw][hp] = True

            # ---- lead-in: QT/KT (pair 0, window 0) pipelined against the
            # arriving xT quarter-0 chunks; fins on ACT (idle before attention)
            lead_qt = pjp.tile([128, 512], F32, tag="pj", name="lead_qt")
            lead_kt = pjp.tile([128, 256], F32, tag="pj", name="lead_kt")
            # consume xT chunks in their expected arrival order (Pool SWDGE
            # k4-7 descriptors generate faster than the SP HWDGE chain).  The
            # KT lead tile covers only kp chunks 0-1 (256 cols) so the first
            # exp fires as soon as the last quarter lands; chunks 2-3 follow
            # as the first filler unit.
            lead_korder = [4, 0, 5, 1, 6, 2, 7, 3]
            for ki, k in enumerate(lead_korder):
                st, sp = (ki == 0), (ki == KC - 1 and not with_bias)
                nc.tensor.matmul(
                    lead_qt[:, :],
                    lhsT=wq_sb[:, k * 128:(k + 1) * 128],
                    rhs=xT_sb[:, k * S: k * S + 512],
                    start=st, stop=sp,
                )
                nc.tensor.matmul(
                    lead_kt[:, :],
                    lhsT=wk_sb[:, k * 128:(k + 1) * 128],
                    rhs=xT_sb[:, k * S: k * S + 256],
                    start=st, stop=sp,
                )
            # QT fin on ACT, KT fin on DVE: both engines idle here, and the
            # first score matmul needs both done.
            qk_fin_w(lead_qt, qt_sb, bq_, 0, 0, 512, on_act=True)
            qk_fin_w(lead_kt, kt_sb, bk_, 0, 0, 256, on_act=False)

            # ---- main loop
            ui = 0           # next filler unit
            pending = []     # FIFO of (hp, qw, c, pt_tile)
            oacc_cur = {}    # (hp,qw) -> (oA, oB)
            drained = set()

            def get_oacc(hp, qw):
                if (hp, qw) not in oacc_cur:
                    oacc_cur[(hp, qw)] = (
                        oap.tile([128, 260], F32, tag="oA", name=f"oA{hp}{qw}"),
                        oap.tile([128, 260], F32, tag="oB", name=f"oB{hp}{qw}"),
                    )
                return oacc_cur[(hp, qw)]

            def pv_head_ready():
                hp_, qw_, c_, _ = pending[0]
                return v_emitted[hp_][c_]

            def force_v(pair, sb):
                """Emit all remaining units of V tile (pair, sb) immediately.
                Out-of-order vs the deadline list is fine for the pj ring as
                long as tile-internal unit order is preserved."""
                j = ui
                while not v_emitted[pair][sb]:
                    assert j < len(units), f"no units left for V({pair},{sb})"
                    u = units[j]
                    if len(u) > 3 and u[3] == ("v", pair, sb):
                        u[1]()
                        units.pop(j)
                    else:
                        j += 1

            def pop_pv(maxn):
                n = 0
                while pending and n < maxn:
                    if not pv_head_ready():
                        break
                    hp_, qw_, c_, pt_ = pending.pop(0)
                    oA, oB = get_oacc(hp_, qw_)
                    emit_pv(hp_, qw_, c_, pt_, oA, oB)
                    n += 1
                    if c_ == SB - 1:
                        emit_drain(hp_, qw_, oA, oB)
                        drained.add((hp_, qw_))
                        del oacc_cur[(hp_, qw_)]

            budget_rate = float(_K('BRATE', 500))
            budget = -2000.0

            slots = [
                (hp, qw, c)
                for hp in range(2) for qw in range(NQW) for c in range(SB)
            ]

            def emit_qk(hp, qw, c, slot):
                # scores for chunk c (both heads packed)
                sc = scp.tile([128, 1024], F32, tag="sc", name=f"s{slot}")
                for i in range(2):
                    nc.tensor.matmul(
                        sc[:, 512 * i:512 * (i + 1)],
                        lhsT=kt_sb[
                            64 * i:64 * (i + 1),
                            hp * S + c * 128: hp * S + (c + 1) * 128,
                        ],
                        rhs=qt_sb[
                            64 * i:64 * (i + 1),
                            hp * S + qw * 512: hp * S + (qw + 1) * 512,
                        ],
                        start=True,
                        stop=True,
                    )
                return sc

            # QK runs one slot ahead of the exp stream (sc is double-
            # buffered), so deadline-forced fillers sit BEHIND the next
            # slot's QK in the PE queue and can never stall the exp cadence
            # by more than one slot of excess.
            sc_cur = None if OLDSCHED else emit_qk(*slots[0], 0)
            for slot, (hp, qw, c) in enumerate(slots):
                if OLDSCHED:
                    sc_cur = emit_qk(hp, qw, c, slot)
                # keep the pt ring from stalling the exp stream
                if len(pending) >= _K('PTCAP', 14):
                    hp_, qw_, c_, _ = pending[0]
                    if not v_emitted[hp_][c_]:
                        force_v(hp_, c_)
                    pop_pv(4)
                pt_t = ptp.tile([128, 1024], BF, tag="pt", name=f"p{slot}")
                nc.scalar.activation(pt_t[:, :], sc_cur[:, :], EXP, scale=0.125)
                pending.append((hp, qw, c, pt_t))
                if not OLDSCHED and slot + 1 < len(slots):
                    sc_cur = emit_qk(*slots[slot + 1], slot + 1)
                if (not OLDSCHED) and hp == 1 and qw >= NQW - 2:
                    # last windows: drain PV eagerly so the post-exp tail
                    # (drain -> transpose -> outproj) starts early
                    pop_pv(3)
                else:
                    pop_pv(2 if len(pending) > _K('POPTH', 6) else 1)
                # fillers by deadline + budget
                budget = min(budget + budget_rate, 900.0)
                while ui < len(units):
                    u = units[ui]
                    if u[0] > slot and budget < u[2]:
                        break
                    if len(u) > 3 and u[3][0] == "op":
                        qw_ = u[3][1]
                        if not (op_ready[qw_][0] and op_ready[qw_][1]):
                            break
                    u[1]()
                    budget -= u[2]
                    ui += 1

            # ---- tail
            while pending:
                if not pv_head_ready():
                    hp_, qw_, c_, _ = pending[0]
                    force_v(hp_, c_)
                pop_pv(99)
            while ui < len(units):
                u = units[ui]
                if len(u) > 3 and u[3][0] == "op":
                    qw_ = u[3][1]
                    assert op_ready[qw_][0] and op_ready[qw_][1]
                u[1]()
                ui += 1

            if debug:
                nc.sync.dma_start(dbg["qt"][:, :], qt_sb[:, :])
                nc.sync.dma_start(dbg["kt"][:, :], kt_sb[:, :])
                nc.sync.dma_start(dbg["v"][:, :], v_sb[:, :])
                nc.sync.dma_start(dbg["onm"][:, :], onm_sb[:, :])
                nc.sync.dma_start(dbg["onmT"][:, :], onmT_sb[:, :])

    nc.compile()
    return nc


def _get_nc(with_bias=False):
    if with_bias not in _nc_cache:
        _nc_cache[with_bias] = _build_bass(with_bias=with_bias)
    return _nc_cache[with_bias]


def _prepare_in_maps(x, wq, bq, wk, bk, wv, bv, wo, with_bias):
    import ml_dtypes

    bf16 = ml_dtypes.bfloat16
    x = np.asarray(x, np.float32)
    wq, bq = np.asarray(wq, np.float32), np.asarray(bq, np.float32)
    wk, bk = np.asarray(wk, np.float32), np.asarray(bk, np.float32)
    wv, bv = np.asarray(wv, np.float32), np.asarray(bv, np.float32)
    wo = np.asarray(wo, np.float32)

    xT = [np.ascontiguousarray(x[b].T).astype(bf16) for b in range(B)]

    def swz(w):
        # [k*128+p, h*128+c] -> [h*128+p, k*128+c]: each output-column half
        # becomes one contiguous [128, 1024] block (single fast DMA).
        return np.ascontiguousarray(
            w.reshape(KC, 128, 2, 128).transpose(2, 1, 0, 3).reshape(DHC, D)
        ).astype(bf16)

    in_maps = []
    for c in range(NC):
        b, j = divmod(c, HPC)
        cs = slice(DHC * j, DHC * (j + 1))
        m = {
            "xT": xT[b],
            "wq_c": swz(wq[:, cs]),
            "wk_c": swz(wk[:, cs]),
            "wv_c": swz(wv[:, cs]),
            "wo_c": np.ascontiguousarray(wo[cs, :]).astype(bf16),
        }
        if with_bias:
            bias3 = np.concatenate([bq[cs], bk[cs], bv[cs]]).reshape(1, 3 * DHC)
            m["bias3"] = np.ascontiguousarray(bias3.astype(bf16))
        in_maps.append(m)
    return in_maps


def kernel(x, wq, bq, wk, bk, wv, bv, wo, bo):
    from concourse import bass_utils

    with_bias = bool(
        np.any(np.asarray(bq)) or np.any(np.asarray(bk)) or np.any(np.asarray(bv))
    )
    in_maps = _prepare_in_maps(x, wq, bq, wk, bk, wv, bv, wo, with_bias)
    res = bass_utils.run_bass_kernel_spmd(
        nc=_get_nc(with_bias), in_maps=in_maps, core_ids=list(range(NC))
    )
    bo = np.asarray(bo, np.float32)
    out = np.empty((B, S, D), np.float32)
    for b in range(B):
        acc = np.asarray(res.results[HPC * b]["out"], np.float32)
        for j in range(1, HPC):
            acc = acc + np.asarray(res.results[HPC * b + j]["out"], np.float32)
        out[b] = acc + bo
    return out
